# revision 1
# baseline (speedup 1.0000x reference)
"""Trainium2 Bass kernel for the BDH-style recurrent block.

Strategy: data-parallel over B (8 batches -> 8 NeuronCores, no collectives).
The T=128-step scan is de-sequentialized into dense matmuls per core:

  u_t = relu(emb_t @ Dx.T)                                  (T,N) batched matmul
  x_t = (XD*x_{t-1} + u_t)/s_t  with s_t = XD + sum(u_t)    (L1 norm; x>=0)
      => x = C @ u, C[t,s] = (1/s_s) exp(A_t - A_s), A_t = cumsum log(XD/s_r)
  a*_t = rho_{t-1} @ x_t = ((DecayMask . X X^T) @ ln(emb))_t   (rho_0 = 0)
  y_t  = relu(ln(a*_t) @ Dy.T) * x_t                        (x_t >= 0)
  v*_t = ln(y_t @ E.T)

Matmuls run in float32r (~1.5e-4 rounding, 4x faster PE streaming at free
dims >= 256). X/Y live in (t, n)-major layout; PE transposes provide the
n-major tiles needed for the Gram matrix and the E contraction. A bf16
dummy-matmul warmup during the initial weight DMA lifts the PE HAM clock
gate to 2.4 GHz before the real work arrives.
"""

import math
from contextlib import ExitStack

import numpy as np

N = 2048
D = 256
B = 8
T = 128
XD = 0.97
UD = 0.97
LN_EPS = 1e-5
L1_EPS = 1e-12

# log-domain recentring: E[sum relu(N(0,1)) over 2048] + XD ~ 818.9
LNC2INV = 6.7065
C2 = math.exp(-LNC2INV)
K1 = LNC2INV - math.log(XD)

KD = D // 128   # 2
KN = N // 128   # 16
NJ = N // 512   # 4
WARMUP_MMS = 8   # 512-col bf16 MMs, ~0.53us each cold: ~4.3us dense warmup
NCONST = 5 * T + 1  # packed const block columns


def _pack_jk(wT):
    # (KD,128,N) k-major -> (128, [j(4), k(2), 512]) per-partition contiguous
    return np.ascontiguousarray(
        wT.reshape(KD, 128, NJ, 512).transpose(1, 2, 0, 3).reshape(128, KD * N))

_cache = {}
SIM_MODE = False  # CoreSim's xorwow random-fill is broken; use memset there


def _consts():
    """Packed (128, NCONST) const block: [utones | trik | dmaskT | ident |
    negblock | xdvec-col]. One contiguous DMA."""
    r = np.arange(T)
    utones = (r[:, None] <= r[None, :]).astype(np.float32)          # [r,t] r<=t
    tri = r[None, :] - r[:, None]                                   # t - s
    trik = np.where(tri >= 0, -K1 * tri - LNC2INV, -10000.0).astype(np.float32)
    pw = r[:, None] - 1 - r[None, :]                                # [t,s] t-1-s
    dmask = np.where(pw >= 0, UD ** np.maximum(pw, 0), 0.0).astype(np.float32)
    dmaskT = np.ascontiguousarray(dmask.T)                          # [s,t]
    ident = np.eye(T, dtype=np.float32)
    negblock = -np.ones((T, T), dtype=np.float32)
    xdvec = np.full((T, 1), C2 * XD, dtype=np.float32)
    xdvec[0, 0] = 0.0                                               # x_{-1} = 0
    return np.ascontiguousarray(np.concatenate(
        [utones, trik, dmaskT, ident, negblock, xdvec], axis=1))


def _split_multiwait(nc, mybir):
    """This walrus build caps sync waits per instruction (1 for regular
    instructions, 2 for EventSemaphore). Tile attaches more (e.g. the
    kernel-tail Drain waits on every live semaphore). Hoist excess waits
    onto same-engine NOPs placed immediately before the instruction —
    engine queues are sequential, so semantics are preserved."""
    n = 0
    for f in nc.m.functions:
        for bb in f.blocks:
            out = []
            changed = False
            for ins in bb.instructions:
                si = ins.sync_info
                ow = list(si.on_wait) if si is not None else []
                cap = 2 if ins.opcode == "EventSemaphore" else 1
                if len(ow) > cap:
                    sem_waits = [w for w in ow if w.sync_type == "semaphore"]
                    other = [w for w in ow if w.sync_type != "semaphore"]
                    keep = max(cap - len(other), 0)
                    hoist = sem_waits[:len(sem_waits) - keep] if keep else sem_waits
                    kept = sem_waits[len(hoist):] + other
                    assert len(kept) <= cap, (len(kept), cap, ins.opcode)
                    changed = True
                    for w in hoist:
                        n += 1
                        nop = mybir.InstNoOp(
                            name=f"wsplit-{n}",
                            sync_info=mybir.SyncInfo(on_wait=[w], on_update=[]),
                            bass_nofuse=True,
                            engine=ins.engine,
                        )
                        nc.register_instruction(nop, overwrite=True)
                        out.append(nop)
                    si.on_wait = kept
                out.append(ins)
            if changed:
                bb.instructions = out
    return nc


def _build():
    import concourse.bass as bass
    import concourse.mybir as mybir
    import concourse.tile as tile

    f32 = mybir.dt.float32
    f32r = mybir.dt.float32r
    bf16 = mybir.dt.bfloat16
    AF = mybir.ActivationFunctionType
    ALU = mybir.AluOpType
    AX = mybir.AxisListType

    from concourse.vector_clock import ScopedClock

    class _TrimTailTC(tile.TileContext):
        # Drop the second kernel-tail all-engine barrier: it only orders
        # the semaphore resets against engine halt, and nothing executes
        # after it. The first barrier (before resets) is kept, so resets
        # still happen on a quiesced machine and re-execution stays safe.
        def _drain_and_barrier(self, tick_clock, wait_clock):
            drain_inst = self.nc.sync.drain()
            wait_clock.add_sem_waits(
                drain_inst.ins, ScopedClock({None: tick_clock.global_clock})
            )
            self.nc.all_engine_barrier()
            assert self.sems is not None
            popped = self.nc._tile_sem_poison_stack.pop()
            assert popped is self._sem_poison
            self.nc.clear_and_free_semaphores(
                list(self.sems.allocated().values())
            )

    nc = bass.Bass()

    d_emb = nc.dram_tensor("emb", [T, D], f32, kind="ExternalInput")
    d_embT = nc.dram_tensor("embT", [128, KD * T], f32, kind="ExternalInput")
    d_dxT = nc.dram_tensor("dxT", [128, KD * N], f32, kind="ExternalInput")
    d_dyT = nc.dram_tensor("dyT", [128, KD * N], f32, kind="ExternalInput")
    d_eT = nc.dram_tensor("eT", [128, KN * D], f32, kind="ExternalInput")
    d_consts = nc.dram_tensor("consts", [128, NCONST], f32, kind="ExternalInput")
    d_out = nc.dram_tensor("out", [T, D], f32, kind="ExternalOutput")

    with _TrimTailTC(nc) as tc, ExitStack() as ctx:
        work = ctx.enter_context(tc.tile_pool(name="work", bufs=1))
        stats = ctx.enter_context(tc.tile_pool(name="stats", bufs=1))
        p_u = ctx.enter_context(tc.tile_pool(name="p_u", bufs=2, space="PSUM"))
        p_sq = ctx.enter_context(tc.tile_pool(name="p_sq", bufs=4, space="PSUM"))
        p_g = ctx.enter_context(tc.tile_pool(name="p_g", bufs=1, space="PSUM"))
        p_med = ctx.enter_context(tc.tile_pool(name="p_med", bufs=1, space="PSUM"))

        # ---- PE warmup: random-data bf16 matmuls while weights stream ---
        # (all-zero operands leave the HAM activity monitor cold: no
        # switching activity -> the clock gate never lifts to 2.4 GHz)
        wu_sb = work.tile([128, 512], bf16)
        if SIM_MODE:
            nc.vector.memset(wu_sb[:], 1.0)
        else:
            nc.vector.random(wu_sb[:])
        wu_ps = p_u.tile([128, 512], f32, tag="pu")
        for i in range(WARMUP_MMS):
            nc.tensor.matmul(wu_ps[:], wu_sb[:, 0:128], wu_sb[:], start=True,
                             stop=True)

        # ---- activation table preloads (Ln/Exp used mid-kernel) ---------
        pre_sb = stats.tile([1, 1], f32)
        nc.vector.memset(pre_sb[:], 1.0)
        pre_o = stats.tile([1, 1], f32)
        nc.scalar.activation(pre_o[:], pre_sb[:], AF.Ln)
        nc.scalar.activation(pre_o[:], pre_sb[:], AF.Exp)
        nc.scalar.activation(pre_o[:], pre_sb[:], AF.Square)

        # ---- DMAs: qSP-HWDGE executes these in FIFO order, each striped
        # across the 16 SDMA engines at ~full HBM rate. Order and piece
        # granularity = the compute pipeline's start schedule.
        # dxT/dyT are packed [j(4), k(2), 512] so consumer j needs only
        # piece j; eT is chunk-major so vraw group g needs piece g.
        embT_sb = work.tile([128, KD * T], f32r)
        nc.sync.dma_start(embT_sb[:], d_embT[:].bitcast(f32r))
        dxT_sb = work.tile([128, KD * N], f32r)
        for j in range(NJ):
            nc.sync.dma_start(dxT_sb[:, j * 1024:(j + 1) * 1024],
                              d_dxT[:, j * 1024:(j + 1) * 1024].bitcast(f32r))
        consts_sb = work.tile([128, NCONST], f32)
        nc.sync.dma_start(consts_sb[:], d_consts[:])
        utones_sb = consts_sb[:, 0:T]
        trik_sb = consts_sb[:, T:2 * T]
        dmaskT_sb = consts_sb[:, 2 * T:3 * T]
        negones_sb = consts_sb[0:1, 4 * T:5 * T]
        xdvec_sb = consts_sb[:, 5 * T:5 * T + 1]
        emb_sb = work.tile([T, D], f32)
        nc.sync.dma_start(emb_sb[:], d_emb[:])
        ident_t = work.tile([T, T], f32r)
        nc.sync.dma_start(ident_t[:], d_consts[:, 3 * T:4 * T].bitcast(f32r))
        ident_sb = ident_t[:]
        dyT_sb = work.tile([128, KD * N], f32r)
        for j in range(NJ):
            nc.sync.dma_start(dyT_sb[:, j * 1024:(j + 1) * 1024],
                              d_dyT[:, j * 1024:(j + 1) * 1024].bitcast(f32r))
        eT_sb = work.tile([128, KN * D], f32r)
        for g in range(4):
            nc.sync.dma_start(eT_sb[:, g * 1024:(g + 1) * 1024],
                              d_eT[:, g * 1024:(g + 1) * 1024].bitcast(f32r))

        def keepalive(ap):
            # PE matmuls gated on a late LN stat: hold the HAM clock warm
            # (with real switching activity) through serial non-PE stretches.
            ka = p_sq.tile([T, T], f32, tag="sq")
            nc.tensor.matmul(ka[0:1, :], ap, trik_sb[:], start=True, stop=True)
            nc.tensor.matmul(ka[0:1, :], ap, dmaskT_sb[:], start=True, stop=True)

        def fast_ln(psum_src, dst, tagp, hold_pe=False):
            """dst = LN(psum_src) straight out of PSUM: bn_stats/bn_aggr for
            mean+var (one DVE pass), short mostly-DVE scalar chain, one ACT
            pass for the normalize. No SBUF evac needed."""
            stat6 = stats.tile([T, 6], f32, tag=f"{tagp}_s6")
            nc.vector.bn_stats(stat6[:], psum_src)
            mv = stats.tile([T, 2], f32, tag=f"{tagp}_mv")
            nc.vector.bn_aggr(mv[:], stat6[:])
            if hold_pe:
                keepalive(mv[:, 0:1])
            veps = stats.tile([T, 1], f32, tag=f"{tagp}_ve")
            nc.vector.tensor_scalar_add(veps[:], mv[:, 1:2], LN_EPS)
            rv = stats.tile([T, 1], f32, tag=f"{tagp}_rv")
            nc.vector.reciprocal(rv[:], veps[:])
            rstd = stats.tile([T, 1], f32, tag=f"{tagp}_rs")
            nc.scalar.sqrt(rstd[:], rv[:])
            if hold_pe:
                keepalive(rstd[:])
            nmr = stats.tile([T, 1], f32, tag=f"{tagp}_nr")
            nc.vector.scalar_tensor_tensor(nmr[:], mv[:, 0:1], -1.0, rstd[:],
                                           op0=ALU.mult, op1=ALU.mult)
            nc.scalar.activation(dst[:], psum_src, AF.Identity,
                                 scale=rstd[:], bias=nmr[:])

        # ---- all-ACT layernorm helper -----------------------------------
        def layernorm(src, dst, tagp, evac=None, hold_pe=False):
            """dst = LN(src) over free dim. If evac is a PSUM AP, src is
            filled from it (evac+rowsum fused); else src must be SBUF and
            a junk copy produces the rowsum."""
            junk = work.tile([T, D], f32, tag="lnjunk")
            msum = stats.tile([T, 1], f32, tag=f"{tagp}_ms")
            if evac is not None:
                nc.scalar.activation(src[:], evac, AF.Copy, accum_out=msum[:])
            else:
                nc.scalar.activation(junk[:], src[:], AF.Copy, accum_out=msum[:])
            negm = stats.tile([T, 1], f32, tag=f"{tagp}_nm")
            nc.scalar.mul(negm[:], msum[:], -1.0 / D)
            if hold_pe:
                keepalive(negm[:])
            ssum = stats.tile([T, 1], f32, tag=f"{tagp}_ss")
            nc.scalar.activation(junk[:], src[:], AF.Square, bias=negm[:],
                                 accum_out=ssum[:])
            veps = stats.tile([T, 1], f32, tag=f"{tagp}_ve")
            nc.vector.tensor_scalar(veps[:], ssum[:], 1.0 / D, LN_EPS,
                                    op0=ALU.mult, op1=ALU.add)
            lv = stats.tile([T, 1], f32, tag=f"{tagp}_lv")
            nc.scalar.activation(lv[:], veps[:], AF.Ln)
            rstd = stats.tile([T, 1], f32, tag=f"{tagp}_rs")
            nc.scalar.activation(rstd[:], lv[:], AF.Exp, scale=-0.5)
            if hold_pe:
                keepalive(rstd[:])
            nmr = stats.tile([T, 1], f32, tag=f"{tagp}_nr")
            nc.scalar.mul(nmr[:], negm[:], rstd[:])
            nc.scalar.activation(dst[:], src[:], AF.Identity,
                                 scale=rstd[:], bias=nmr[:])

        # ---- u = relu(emb @ Dx.T) (f32r), row sums ----------------------
        u_sb = work.tile([T, N], f32r)
        su_part = stats.tile([T, NJ], f32)
        for j in range(NJ):
            ps = p_u.tile([128, 512], f32, tag="pu")
            for c in range(KD):
                nc.tensor.matmul(
                    ps[:],
                    embT_sb[:, c * T:(c + 1) * T],
                    dxT_sb[:, j * 1024 + c * 512: j * 1024 + (c + 1) * 512],
                    start=(c == 0),
                    stop=(c == KD - 1),
                )
            nc.scalar.activation(
                u_sb[:, j * 512:(j + 1) * 512], ps[:], AF.Relu,
                accum_out=su_part[:, j:j + 1],
            )

        # ---- C^T coefficient matrix -------------------------------------
        su = stats.tile([T, 1], f32)
        nc.vector.tensor_reduce(su[:], su_part[:], axis=AX.X, op=ALU.add)
        keepalive(su[:])
        q_sb = stats.tile([T, 1], f32)
        nc.scalar.activation(q_sb[:], su[:], AF.Ln, scale=C2, bias=xdvec_sb[:])

        qc = p_sq.tile([T, T], f32, tag="sq")               # Q_s column
        nc.tensor.matmul(qc[:, 0:1], utones_sb[:], q_sb[:], start=True, stop=True)
        qr = p_sq.tile([T, T], f32, tag="sq")               # Q_t row
        nc.tensor.matmul(qr[0:1, :], q_sb[:], utones_sb[:], start=True, stop=True)
        qr_sb = stats.tile([1, T], f32)
        nc.vector.tensor_copy(qr_sb[:], qr[0:1, :])
        colsc = stats.tile([T, 1], f32)                     # Q_s - q_s
        nc.vector.tensor_sub(colsc[:], qc[:, 0:1], q_sb[:])
        keepalive(colsc[:])
        bc = p_sq.tile([T, T], f32, tag="sq")               # [s,t] = -Q_t
        nc.tensor.matmul(bc[:], negones_sb[:], qr_sb[:], start=True, stop=True)

        expo = work.tile([T, T], f32)
        nc.vector.scalar_tensor_tensor(
            expo[:], bc[:], colsc[:], trik_sb[:], op0=ALU.add, op1=ALU.add
        )
        expoc = work.tile([T, T], f32)
        nc.vector.tensor_scalar_max(expoc[:], expo[:], -80.0)
        ct_sb = work.tile([T, T], f32r)                     # C^T [s,t]
        nc.scalar.activation(ct_sb[:], expoc[:], AF.Exp)

        # ---- vn = LN(emb) (off critical path) ---------------------------
        vn_sb = work.tile([T, D], f32r)
        fast_ln(emb_sb[:], vn_sb, "vn")

        # ---- X = C @ u (t,n-major, f32r), X^T via PE transpose ----------
        x_sb = work.tile([T, N], f32r)
        for j in range(NJ):
            ps = p_u.tile([128, 512], f32, tag="pu")
            nc.tensor.matmul(ps[:], ct_sb[:], u_sb[:, j * 512:(j + 1) * 512],
                             start=True, stop=True)
            if j % 2 == 0:
                nc.vector.tensor_copy(x_sb[:, j * 512:(j + 1) * 512], ps[:])
            else:
                nc.scalar.copy(x_sb[:, j * 512:(j + 1) * 512], ps[:])

        # ---- X^T via PE transpose, G = X X^T, interleaved per chunk -----
        xt_sb = work.tile([128, N], f32r)
        g = p_g.tile([T, T], f32, tag="g")
        for gq in range(4):
            for cc in range(4):
                c = 4 * gq + cc
                tp = p_sq.tile([T, T], f32, tag="sq")
                nc.tensor.transpose(tp[:].bitcast(f32r),
                                    x_sb[:, c * T:(c + 1) * T], ident_sb)
                if c % 2 == 0:
                    nc.vector.tensor_copy(xt_sb[:, c * T:(c + 1) * T], tp[:])
                else:
                    nc.scalar.copy(xt_sb[:, c * T:(c + 1) * T], tp[:])
            for cc in range(4):
                c = 4 * gq + cc
                nc.tensor.matmul(g[:], xt_sb[:, c * T:(c + 1) * T],
                                 xt_sb[:, c * T:(c + 1) * T],
                                 start=(c == 0), stop=(c == KN - 1))
        wt_sb = work.tile([T, T], f32r)
        nc.vector.tensor_mul(wt_sb[:], g[:], dmaskT_sb[:])

        # ---- a* = W @ vn, LN, transpose ---------------------------------
        aps = p_med.tile([T, D], f32, tag="med")
        nc.tensor.matmul(aps[:], wt_sb[:], vn_sb[:], start=True, stop=True)
        lna_sb = work.tile([T, D], f32r)
        fast_ln(aps[:], lna_sb, "la", hold_pe=True)

        lnaT_sb = work.tile([128, KD * T], f32r)
        for c in range(KD):
            tp = p_sq.tile([T, T], f32, tag="sq")
            nc.tensor.transpose(tp[:].bitcast(f32r), lna_sb[:, c * T:(c + 1) * T],
                                ident_sb)
            nc.scalar.copy(lnaT_sb[:, c * T:(c + 1) * T], tp[:])

        # ---- Ycore (t,n-major, f32r), Y = relu(Ycore) * X ---------------
        # ---- Ycore -> Y -> Y^T -> v_raw, interleaved per j-group --------
        y_sb = work.tile([T, N], f32r)
        yt_sb = work.tile([128, N], f32r)
        vps = p_med.tile([T, D], f32, tag="med")
        for j in range(NJ):
            ps = p_u.tile([128, 512], f32, tag="pu")
            for k in range(KD):
                nc.tensor.matmul(ps[:], lnaT_sb[:, k * T:(k + 1) * T],
                                 dyT_sb[:, j * 1024 + k * 512: j * 1024 + (k + 1) * 512],
                                 start=(k == 0), stop=(k == KD - 1))
            nc.vector.scalar_tensor_tensor(
                y_sb[:, j * 512:(j + 1) * 512], ps[:], 0.0,
                x_sb[:, j * 512:(j + 1) * 512], op0=ALU.max, op1=ALU.mult,
            )
            # transposes+evacs first, then the vraw MMs: the PE executes its
            # queue in order, so this hides each evac under later transposes
            for cc in range(4):
                c = 4 * j + cc
                tp = p_sq.tile([T, T], f32, tag="sq")
                nc.tensor.transpose(tp[:].bitcast(f32r),
                                    y_sb[:, c * T:(c + 1) * T], ident_sb)
                if c % 2 == 0:
                    nc.vector.tensor_copy(yt_sb[:, c * T:(c + 1) * T], tp[:])
                else:
                    nc.scalar.copy(yt_sb[:, c * T:(c + 1) * T], tp[:])
            for cc in range(4):
                c = 4 * j + cc
                nc.tensor.matmul(vps[:], yt_sb[:, c * T:(c + 1) * T],
                                 eT_sb[:, c * D:(c + 1) * D],
                                 start=(c == 0), stop=(c == KN - 1))
        vstar_sb = work.tile([T, D], f32)
        fast_ln(vps[:], vstar_sb, "vs")

        nc.sync.dma_start(d_out[:], vstar_sb[:])

    return _split_multiwait(nc, mybir)


def _numpy_fallback(embeddings, E, Dx, Dy, x_state, rho_state):
    # General-path reference (only used if initial states are nonzero).
    def ln(x):
        m = x.mean(-1, keepdims=True)
        v = ((x - m) ** 2).mean(-1, keepdims=True)
        return (x - m) / np.sqrt(v + LN_EPS)

    x_s = x_state.astype(np.float32).copy()
    rho = rho_state.astype(np.float32).copy()
    outs = np.zeros((B, T, D), dtype=np.float32)
    for t in range(T):
        v_prev = embeddings[:, t, :]
        x_upd = np.maximum(v_prev @ Dx.T, 0.0)
        x_t = XD * x_s + x_upd
        x_t = x_t / np.maximum(np.abs(x_t).sum(-1, keepdims=True), L1_EPS)
        a_star = np.einsum("bdn,bn->bd", rho, x_t)
        y_core = ln(a_star) @ Dy.T
        y_t = np.maximum(y_core, 0.0) * np.maximum(x_t, 0.0)
        outs[:, t, :] = ln(y_t @ E.T)
        vn = ln(v_prev)
        rho = UD * rho + np.einsum("bd,bn->bdn", vn, x_t)
        x_s = x_t
    return outs


def kernel(embeddings, E, Dx, Dy, x_state, rho_state):
    embeddings = np.ascontiguousarray(embeddings, dtype=np.float32)
    E = np.ascontiguousarray(E, dtype=np.float32)
    Dx = np.ascontiguousarray(Dx, dtype=np.float32)
    Dy = np.ascontiguousarray(Dy, dtype=np.float32)

    if np.any(x_state) or np.any(rho_state):
        return _numpy_fallback(embeddings, E, Dx, Dy,
                               np.asarray(x_state, np.float32),
                               np.asarray(rho_state, np.float32))

    from concourse.bass_utils import run_bass_kernel_spmd

    if "nc" not in _cache:
        _cache["nc"] = _build()
    nc = _cache["nc"]

    consts = _consts()
    # SBUF-layout packing: row p holds that partition's contiguous span.
    dxT = _pack_jk(Dx.T.reshape(KD, 128, N))
    dyT = _pack_jk(Dy.T.reshape(KD, 128, N))
    eT = np.ascontiguousarray(
        E.T.reshape(KN, 128, D).transpose(1, 0, 2).reshape(128, KN * D))

    in_maps = []
    for b in range(B):
        emb_b = embeddings[b]
        embT_b = np.ascontiguousarray(
            emb_b.T.reshape(KD, 128, T).transpose(1, 0, 2).reshape(128, KD * T))
        in_maps.append({
            "emb": emb_b,
            "embT": embT_b,
            "dxT": dxT,
            "dyT": dyT,
            "eT": eT,
            "consts": consts,
        })

    res = run_bass_kernel_spmd(nc, in_maps, list(range(B)))
    _cache["last_results"] = res
    return np.stack([res.results[i]["out"] for i in range(B)])



# revision 5
# speedup vs baseline: 1.1984x; 1.1984x over previous
"""Trainium2 Bass kernel for the BDH-style recurrent block.

Strategy: data-parallel over B (8 batches -> 8 NeuronCores, no collectives).
The T=128-step scan is de-sequentialized into dense matmuls per core:

  u_t = relu(emb_t @ Dx.T)                                  (T,N)
  x_t = (XD*x_{t-1} + u_t)/s_t  with s_t = XD + sum(u_t)    (L1 norm; x>=0)
      => X = C @ u, C[t,s] = (1/c_s) exp(A_t - A_s), A_t = cumsum log(XD/c_r)
  a*_t = rho_{t-1} @ x_t = ((DecayMask . X X^T) @ ln(emb))_t   (rho_0 = 0)
  y_t  = relu(ln(a*_t) @ Dy.T) * x_t                        (x_t >= 0)
  v*_t = ln(y_t @ E.T)

All matmuls run in bf16 (1 col/cycle at any free dim, vs f32r's 4 cyc/row
below 256 free): X^T and Ycore^T are produced directly in n-major chunks
(lhsT = u chunk / DyT chunk), so no PE transposes of X/Y are needed.
LayerNorm rstd uses exp(-0.5*ln(v+eps)) so the only ACT table set ever
needed is natural_log_exp_and_others -> one table load, at kernel start.
Inputs arrive as one packed bf16 blob + a small f32 const tensor, posted
in need-order across the Sync and Scalar HWDGE queues.
"""

import math
from contextlib import ExitStack

import numpy as np

N = 2048
D = 256
B = 8
T = 128
XD = 0.97
UD = 0.97
LN_EPS = 1e-5

# log-domain recentring: E[sum relu(N(0,1)) over 2048] + XD ~ 818.9
LNC2INV = 6.7065
C2 = math.exp(-LNC2INV)
K1 = LNC2INV - math.log(XD)

KD = D // 128    # 2
KN = N // 128    # 16
NJ = N // 512    # 4
WARMUP_MMS = 6   # 512-col bf16 MMs while the first DMA piece streams

# bf16 blob column layout (per 128-partition row)
C_UT = 0                 # utones  [s<=t]
C_US = C_UT + T          # ustrict [s< t]
C_ID = C_US + T          # identity
C_DM = C_ID + T          # dmaskT  UD^(t-1-s)
C_EMBT = C_DM + T        # embT (KD*T)
C_EMB = C_EMBT + KD * T  # emb  (D)
C_DX = C_EMB + D         # dxT packed [j(4), k(2), 512]
C_DY = C_DX + KD * N     # dyT packed [c(16), k(2), 128]
C_ET = C_DY + KD * N     # eT  packed [c(16), 256]
CB = C_ET + KN * D


def _consts_bf16():
    import ml_dtypes
    r = np.arange(T)
    utones = (r[:, None] <= r[None, :]).astype(np.float32)
    ustrict = (r[:, None] < r[None, :]).astype(np.float32)
    ident = np.eye(T, dtype=np.float32)
    pw = r[:, None] - 1 - r[None, :]                        # [t,s] t-1-s
    dmask = np.where(pw >= 0, UD ** np.maximum(pw, 0), 0.0).astype(np.float32)
    dmaskT = np.ascontiguousarray(dmask.T)                  # [s,t]
    return np.concatenate([utones, ustrict, ident, dmaskT],
                          axis=1).astype(ml_dtypes.bfloat16)


def _consts_f32():
    r = np.arange(T)
    tri = r[None, :] - r[:, None]                           # t - s
    trik = np.where(tri >= 0, -K1 * tri - LNC2INV, -10000.0).astype(np.float32)
    xdvec = np.full((T, 1), C2 * XD, dtype=np.float32)
    xdvec[0, 0] = 0.0                                       # x_{-1} = 0
    return np.ascontiguousarray(np.concatenate([trik, xdvec], axis=1))


_cache = {}


def _split_multiwait(nc, mybir):
    """This walrus build caps sync waits per instruction (1 for regular
    instructions, 2 for EventSemaphore). Tile attaches more (e.g. the
    kernel-tail Drain waits on every live semaphore). Hoist excess waits
    onto same-engine NOPs placed immediately before the instruction —
    engine queues are sequential, so semantics are preserved."""
    n = 0
    for f in nc.m.functions:
        for bb in f.blocks:
            out = []
            changed = False
            for ins in bb.instructions:
                si = ins.sync_info
                ow = list(si.on_wait) if si is not None else []
                cap = 2 if ins.opcode == "EventSemaphore" else 1
                if len(ow) > cap:
                    sem_waits = [w for w in ow if w.sync_type == "semaphore"]
                    other = [w for w in ow if w.sync_type != "semaphore"]
                    keep = max(cap - len(other), 0)
                    hoist = sem_waits[:len(sem_waits) - keep] if keep else sem_waits
                    kept = sem_waits[len(hoist):] + other
                    assert len(kept) <= cap, (len(kept), cap, ins.opcode)
                    changed = True
                    for w in hoist:
                        n += 1
                        nop = mybir.InstNoOp(
                            name=f"wsplit-{n}",
                            sync_info=mybir.SyncInfo(on_wait=[w], on_update=[]),
                            bass_nofuse=True,
                            engine=ins.engine,
                        )
                        nc.register_instruction(nop, overwrite=True)
                        out.append(nop)
                    si.on_wait = kept
                out.append(ins)
            if changed:
                bb.instructions = out
    return nc


def _build():
    import concourse.bass as bass
    import concourse.mybir as mybir
    import concourse.tile as tile

    f32 = mybir.dt.float32
    bf16 = mybir.dt.bfloat16
    AF = mybir.ActivationFunctionType
    ALU = mybir.AluOpType
    AX = mybir.AxisListType

    from concourse.vector_clock import ScopedClock

    class _TrimTailTC(tile.TileContext):
        # Drop the second kernel-tail all-engine barrier: it only orders
        # the semaphore resets against engine halt, and nothing executes
        # after it. The first barrier (before resets) is kept, so resets
        # still happen on a quiesced machine and re-execution stays safe.
        def _drain_and_barrier(self, tick_clock, wait_clock):
            drain_inst = self.nc.sync.drain()
            wait_clock.add_sem_waits(
                drain_inst.ins, ScopedClock({None: tick_clock.global_clock})
            )
            self.nc.all_engine_barrier()
            assert self.sems is not None
            popped = self.nc._tile_sem_poison_stack.pop()
            assert popped is self._sem_poison
            self.nc.clear_and_free_semaphores(
                list(self.sems.allocated().values())
            )

    nc = bass.Bass()

    d_blob = nc.dram_tensor("blob", [128, CB], bf16, kind="ExternalInput")
    d_cf32 = nc.dram_tensor("cf32", [128, T + 1], f32, kind="ExternalInput")
    d_out = nc.dram_tensor("out", [T, D], f32, kind="ExternalOutput")

    with _TrimTailTC(nc) as tc, ExitStack() as ctx:
        work = ctx.enter_context(tc.tile_pool(name="work", bufs=1))
        stats = ctx.enter_context(tc.tile_pool(name="stats", bufs=1))
        # PSUM slots are bank-granular: 8 banks of [128,512]xf32 total.
        # Tags: pu(2) u-phase, ch(3) xt/tpb/yc rotation, g(1) g->aps->vps,
        # cc(2) qc/qr/bc/keepalive rotation.
        p_ps = ctx.enter_context(tc.tile_pool(name="p_ps", bufs=1, space="PSUM"))

        # ---- DMAs first: HWDGE FIFO order per queue = need order. -------
        blob_sb = work.tile([128, CB], bf16)
        cf32_sb = work.tile([128, T + 1], f32)
        P0 = C_DX + 2048                   # consts+embT+emb+dxT j01
        P1 = C_DX + 4096                   # dxT j23
        nc.sync.dma_start(blob_sb[:, 0:P0], d_blob[:, 0:P0])
        nc.sync.dma_start(blob_sb[:, P0:P1], d_blob[:, P0:P1])
        nc.sync.dma_start(blob_sb[:, C_DY:C_ET], d_blob[:, C_DY:C_ET])
        nc.sync.dma_start(blob_sb[:, C_ET:CB], d_blob[:, C_ET:CB])
        nc.scalar.dma_start(cf32_sb[:], d_cf32[:])

        utones_sb = blob_sb[:, C_UT:C_UT + T]
        ustrict_sb = blob_sb[:, C_US:C_US + T]
        ident_sb = blob_sb[:, C_ID:C_ID + T]
        dmaskT_sb = blob_sb[:, C_DM:C_DM + T]
        embT_sb = blob_sb[:, C_EMBT:C_EMBT + KD * T]
        emb_sb = blob_sb[:, C_EMB:C_EMB + D]
        dxT_sb = blob_sb[:, C_DX:C_DX + KD * N]
        dyT_sb = blob_sb[:, C_DY:C_DY + KD * N]
        eT_sb = blob_sb[:, C_ET:C_ET + KN * D]
        trik_sb = cf32_sb[:, 0:T]
        xdvec_sb = cf32_sb[:, T:T + 1]

        # ---- ACT table preload: Ln+Exp share one set; everything else the
        # kernel uses (Relu/Copy/Identity) is a filler in every set.
        pre_sb = stats.tile([1, 1], f32)
        nc.vector.memset(pre_sb[:], 1.0)
        pre_o = stats.tile([1, 1], f32)
        nc.scalar.activation(pre_o[:], pre_sb[:], AF.Ln)
        nc.scalar.activation(pre_o[:], pre_sb[:], AF.Exp)

        # ---- PE warmup: random-data bf16 matmuls while inputs stream ----
        # (all-zero operands leave the HAM activity monitor cold)
        wu_sb = work.tile([128, 512], bf16)
        nc.vector.random(wu_sb[:])
        wu_ps = p_ps.tile([128, 512], f32, tag="pu", bufs=2)
        for _ in range(WARMUP_MMS):
            nc.tensor.matmul(wu_ps[:], wu_sb[:, 0:128], wu_sb[:], start=True,
                             stop=True)

        def keepalive(ap):
            # PE matmuls gated on a late chain value: hold the HAM clock
            # warm through serial non-PE stretches. ap must be f32 [T,1].
            ka = p_ps.tile([T, T], f32, tag="cc", bufs=2)
            nc.tensor.matmul(ka[0:1, :], ap, trik_sb[:], start=True, stop=True)
            nc.tensor.matmul(ka[0:1, :], ap, trik_sb[:], start=True, stop=True)

        def ln_stats(src, tagp, hold_pe=False):
            """rstd/nmr for LN over the free dim. rstd = exp(-0.5*ln(v+eps))
            keeps everything in the natural_log_exp table set."""
            stat6 = stats.tile([T, 6], f32, tag=f"{tagp}_s6")
            nc.vector.bn_stats(stat6[:], src)
            mv = stats.tile([T, 2], f32, tag=f"{tagp}_mv")
            nc.vector.bn_aggr(mv[:], stat6[:])
            if hold_pe:
                keepalive(mv[:, 0:1])
            veps = stats.tile([T, 1], f32, tag=f"{tagp}_ve")
            nc.vector.tensor_scalar_add(veps[:], mv[:, 1:2], LN_EPS)
            lv = stats.tile([T, 1], f32, tag=f"{tagp}_lv")
            nc.scalar.activation(lv[:], veps[:], AF.Ln)
            rstd = stats.tile([T, 1], f32, tag=f"{tagp}_rs")
            nc.scalar.activation(rstd[:], lv[:], AF.Exp, scale=-0.5)
            if hold_pe:
                keepalive(rstd[:])
            nmr = stats.tile([T, 1], f32, tag=f"{tagp}_nr")
            nc.vector.scalar_tensor_tensor(nmr[:], mv[:, 0:1], -1.0, rstd[:],
                                           op0=ALU.mult, op1=ALU.mult)
            return rstd, nmr

        # ---- u = relu(emb @ Dx.T) (bf16), row sums --------------------
        u_sb = work.tile([T, N], bf16)
        su_part = stats.tile([T, NJ], f32)
        for j in range(NJ):
            ps = p_ps.tile([128, 512], f32, tag="pu", bufs=2)
            for k in range(KD):
                nc.tensor.matmul(
                    ps[:],
                    embT_sb[:, k * T:(k + 1) * T],
                    dxT_sb[:, j * 1024 + k * 512: j * 1024 + (k + 1) * 512],
                    start=(k == 0),
                    stop=(k == KD - 1),
                )
            if j % 2 == 0:
                nc.scalar.activation(
                    u_sb[:, j * 512:(j + 1) * 512], ps[:], AF.Relu,
                    accum_out=su_part[:, j:j + 1],
                )
            else:
                nc.vector.tensor_scalar(
                    u_sb[:, j * 512:(j + 1) * 512], ps[:], 0.0, 0.0,
                    op0=ALU.max, op1=ALU.add,
                    accum_out=su_part[:, j:j + 1],
                )

        # ---- C^T coefficient matrix ------------------------------------
        su = stats.tile([T, 1], f32)
        nc.vector.tensor_reduce(su[:], su_part[:], axis=AX.X, op=ALU.add)
        keepalive(su[:])
        q_sb = stats.tile([T, 1], bf16)
        nc.scalar.activation(q_sb[:], su[:], AF.Ln, scale=C2, bias=xdvec_sb)

        qc = p_ps.tile([T, T], f32, tag="cc", bufs=2)     # Q_{s-1} column
        nc.tensor.matmul(qc[:, 0:1], ustrict_sb, q_sb[:], start=True, stop=True)
        colsc = stats.tile([T, 1], f32)
        nc.vector.tensor_copy(colsc[:], qc[:, 0:1])
        qr = p_ps.tile([T, T], f32, tag="cc", bufs=2)     # Q_t row
        nc.tensor.matmul(qr[0:1, :], q_sb[:], utones_sb, start=True, stop=True)
        qr_sb = stats.tile([1, T], bf16)
        nc.scalar.copy(qr_sb[:], qr[0:1, :])
        bc = p_ps.tile([T, T], f32, tag="cc", bufs=2)     # [s,t] = Q_t
        nc.tensor.matmul(bc[:], utones_sb[0:1, :], qr_sb[:], start=True,
                         stop=True)

        expo = work.tile([T, T], f32)
        nc.vector.scalar_tensor_tensor(
            expo[:], trik_sb[:], colsc[:], bc[:], op0=ALU.add,
            op1=ALU.subtract,
        )
        expoc = work.tile([T, T], f32)
        nc.vector.tensor_scalar_max(expoc[:], expo[:], -80.0)
        ct_sb = work.tile([T, T], bf16)               # C^T [s,t]
        nc.scalar.activation(ct_sb[:], expoc[:], AF.Exp)

        # ---- vn = LN(emb) (off critical path) --------------------------
        vn_sb = work.tile([T, D], bf16)
        r_vn, n_vn = ln_stats(emb_sb, "vn")
        nc.scalar.activation(vn_sb[:], emb_sb, AF.Identity, scale=r_vn[:],
                             bias=n_vn[:])

        # ---- X^T chunks = u_c @ C^T; G = X X^T, interleaved ------------
        xt_sb = work.tile([128, N], bf16)
        g = p_ps.tile([T, T], f32, tag="g", bufs=1)

        def xt_mm(c):
            tp = p_ps.tile([128, T], f32, tag="ch", bufs=3)
            nc.tensor.matmul(tp[:], u_sb[:, c * T:(c + 1) * T], ct_sb[:],
                             start=True, stop=True)
            if c % 2 == 0:
                nc.vector.tensor_copy(xt_sb[:, c * T:(c + 1) * T], tp[:])
            else:
                nc.scalar.copy(xt_sb[:, c * T:(c + 1) * T], tp[:])

        for c in range(KN):
            xt_mm(c)
            if c >= 2:
                cg = c - 2
                nc.tensor.matmul(g[:], xt_sb[:, cg * T:(cg + 1) * T],
                                 xt_sb[:, cg * T:(cg + 1) * T],
                                 start=(cg == 0), stop=False)
        for cg in (KN - 2, KN - 1):
            nc.tensor.matmul(g[:], xt_sb[:, cg * T:(cg + 1) * T],
                             xt_sb[:, cg * T:(cg + 1) * T],
                             start=False, stop=(cg == KN - 1))

        # ---- a* = (G . dmask)^T @ vn, LN, transpose --------------------
        wt_sb = work.tile([T, T], bf16)
        nc.vector.tensor_mul(wt_sb[:], g[:], dmaskT_sb)
        aps = p_ps.tile([T, D], f32, tag="g", bufs=1)
        nc.tensor.matmul(aps[:], wt_sb[:], vn_sb[:], start=True, stop=True)
        r_a, n_a = ln_stats(aps[:], "la", hold_pe=True)
        lna_sb = work.tile([T, D], bf16)
        nc.scalar.activation(lna_sb[:], aps[:], AF.Identity, scale=r_a[:],
                             bias=n_a[:])

        lnaT_sb = work.tile([128, KD * T], bf16)
        for k in range(KD):
            tp = p_ps.tile([128, T], bf16, tag="ch", bufs=3)
            nc.tensor.transpose(tp[:], lna_sb[:, k * T:(k + 1) * T], ident_sb)
            if k % 2 == 0:
                nc.vector.tensor_copy(lnaT_sb[:, k * T:(k + 1) * T], tp[:])
            else:
                nc.scalar.copy(lnaT_sb[:, k * T:(k + 1) * T], tp[:])

        # ---- Ycore^T chunks -> Y^T = relu(.)*X^T -> v accumulation -----
        yt_sb = work.tile([128, N], bf16)
        vps = p_ps.tile([T, D], f32, tag="g", bufs=1)

        def yc_mm(c):
            yc = p_ps.tile([128, T], f32, tag="ch", bufs=3)
            for k in range(KD):
                nc.tensor.matmul(
                    yc[:],
                    dyT_sb[:, c * 256 + k * 128: c * 256 + (k + 1) * 128],
                    lnaT_sb[:, k * T:(k + 1) * T],
                    start=(k == 0), stop=(k == KD - 1),
                )
            nc.vector.scalar_tensor_tensor(
                yt_sb[:, c * T:(c + 1) * T], yc[:], 0.0,
                xt_sb[:, c * T:(c + 1) * T], op0=ALU.max, op1=ALU.mult,
            )

        for c in range(KN):
            yc_mm(c)
            if c >= 2:
                cv = c - 2
                nc.tensor.matmul(vps[:], yt_sb[:, cv * T:(cv + 1) * T],
                                 eT_sb[:, cv * D:(cv + 1) * D],
                                 start=(cv == 0), stop=False)
        for cv in (KN - 2, KN - 1):
            nc.tensor.matmul(vps[:], yt_sb[:, cv * T:(cv + 1) * T],
                             eT_sb[:, cv * D:(cv + 1) * D],
                             start=False, stop=(cv == KN - 1))

        # ---- v* = LN(vps) -> out ---------------------------------------
        r_v, n_v = ln_stats(vps[:], "vs")
        vstar_sb = work.tile([T, D], f32)
        nc.scalar.activation(vstar_sb[:], vps[:], AF.Identity, scale=r_v[:],
                             bias=n_v[:])
        nc.sync.dma_start(d_out[:], vstar_sb[:])

    return _split_multiwait(nc, mybir)


def _numpy_fallback(embeddings, E, Dx, Dy, x_state, rho_state):
    # General-path reference (only used if initial states are nonzero).
    def ln(x):
        m = x.mean(-1, keepdims=True)
        v = ((x - m) ** 2).mean(-1, keepdims=True)
        return (x - m) / np.sqrt(v + LN_EPS)

    x_s = x_state.astype(np.float32).copy()
    rho = rho_state.astype(np.float32).copy()
    outs = np.zeros((B, T, D), dtype=np.float32)
    for t in range(T):
        v_prev = embeddings[:, t, :]
        x_upd = np.maximum(v_prev @ Dx.T, 0.0)
        x_t = XD * x_s + x_upd
        x_t = x_t / np.maximum(np.abs(x_t).sum(-1, keepdims=True), 1e-12)
        a_star = np.einsum("bdn,bn->bd", rho, x_t)
        y_core = ln(a_star) @ Dy.T
        y_t = np.maximum(y_core, 0.0) * np.maximum(x_t, 0.0)
        outs[:, t, :] = ln(y_t @ E.T)
        vn = ln(v_prev)
        rho = UD * rho + np.einsum("bd,bn->bdn", vn, x_t)
        x_s = x_t
    return outs


def kernel(embeddings, E, Dx, Dy, x_state, rho_state):
    import ml_dtypes

    embeddings = np.ascontiguousarray(embeddings, dtype=np.float32)
    E = np.ascontiguousarray(E, dtype=np.float32)
    Dx = np.ascontiguousarray(Dx, dtype=np.float32)
    Dy = np.ascontiguousarray(Dy, dtype=np.float32)

    if np.any(x_state) or np.any(rho_state):
        return _numpy_fallback(embeddings, E, Dx, Dy,
                               np.asarray(x_state, np.float32),
                               np.asarray(rho_state, np.float32))

    from concourse.bass_utils import run_bass_kernel_spmd

    if "nc" not in _cache:
        _cache["nc"] = _build()
    nc = _cache["nc"]

    bf = ml_dtypes.bfloat16
    consts_bf = _consts_bf16()
    cf32 = _consts_f32()
    # SBUF-layout packing: row p holds that partition's contiguous span.
    dxT = np.ascontiguousarray(
        Dx.T.reshape(KD, 128, NJ, 512).transpose(1, 2, 0, 3).reshape(128, KD * N)
    ).astype(bf)
    dyT = np.ascontiguousarray(
        Dy.T.reshape(KD, 128, KN, 128).transpose(1, 2, 0, 3).reshape(128, KD * N)
    ).astype(bf)
    eT = np.ascontiguousarray(
        E.T.reshape(KN, 128, D).transpose(1, 0, 2).reshape(128, KN * D)
    ).astype(bf)

    in_maps = []
    for b in range(B):
        emb_b = embeddings[b]
        embT_b = np.ascontiguousarray(
            emb_b.T.reshape(KD, 128, T).transpose(1, 0, 2).reshape(128, KD * T)
        ).astype(bf)
        blob = np.concatenate(
            [consts_bf, embT_b, emb_b.astype(bf), dxT, dyT, eT], axis=1)
        assert blob.shape == (128, CB), blob.shape
        in_maps.append({"blob": np.ascontiguousarray(blob), "cf32": cf32})

    res = run_bass_kernel_spmd(nc, in_maps, list(range(B)))
    _cache["last_results"] = res
    return np.stack([res.results[i]["out"] for i in range(B)])


# revision 8
# speedup vs baseline: 1.3010x; 1.0856x over previous
"""Trainium2 Bass kernel for the BDH-style recurrent block.

Strategy: data-parallel over B (8 batches -> 8 NeuronCores, no collectives).
The T=128-step scan is de-sequentialized into dense matmuls per core:

  u_t = relu(emb_t @ Dx.T)                                  (T,N)
  x_t = (XD*x_{t-1} + u_t)/s_t  with s_t = XD + sum(u_t)    (L1 norm; x>=0)
      => X = C @ u, C[t,s] = (1/c_s) exp(A_t - A_s), A_t = cumsum log(XD/c_r)
  a*_t = rho_{t-1} @ x_t = ((DecayMask . X X^T) @ ln(emb))_t   (rho_0 = 0)
  y_t  = relu(ln(a*_t) @ Dy.T) * x_t                        (x_t >= 0)
  v*_t = ln(y_t @ E.T)

All matmuls run in bf16 (1 col/cycle at any free dim, vs f32r's 4 cyc/row
below 256 free): X^T and Ycore^T are produced directly in n-major chunks
(lhsT = u chunk / DyT chunk), so no PE transposes of X/Y are needed.
LayerNorm rstd uses exp(-0.5*ln(v+eps)) so the only ACT table set ever
needed is natural_log_exp_and_others -> one table load, at kernel start.
Inputs arrive as one packed bf16 blob + a small f32 const tensor, posted
in need-order across the Sync and Scalar HWDGE queues.
"""

import math
from contextlib import ExitStack

import numpy as np

N = 2048
D = 256
B = 8
T = 128
XD = 0.97
UD = 0.97
LN_EPS = 1e-5

# log-domain recentring: E[sum relu(N(0,1)) over 2048] + XD ~ 818.9
LNC2INV = 6.7065
C2 = math.exp(-LNC2INV)
K1 = LNC2INV - math.log(XD)

KD = D // 128    # 2
KN = N // 128    # 16
NJ = N // 512    # 4
WARMUP_MMS = 14  # 128-col bf16 MMs while the first DMA piece streams

# bf16 blob column layout (per 128-partition row)
C_US = 0                 # ustrict [s< t]
C_ID = C_US + T          # identity
C_DM = C_ID + T          # dmaskT  UD^(t-1-s)
C_EMBT = C_DM + T        # embT (KD*T)
C_EMB = C_EMBT + KD * T  # emb  (D)
C_DX = C_EMB + D         # dxT packed [j(4), k(2), 512]
C_DY = C_DX + KD * N     # dyT packed [c(16), k(2), 128]
C_ET = C_DY + KD * N     # eT  packed [c(16), 256]
CB = C_ET + KN * D


def _consts_bf16():
    import ml_dtypes
    r = np.arange(T)
    ustrict = (r[:, None] < r[None, :]).astype(np.float32)
    ident = np.eye(T, dtype=np.float32)
    pw = r[:, None] - 1 - r[None, :]                        # [t,s] t-1-s
    dmask = np.where(pw >= 0, UD ** np.maximum(pw, 0), 0.0).astype(np.float32)
    dmaskT = np.ascontiguousarray(dmask.T)                  # [s,t]
    return np.concatenate([ustrict, ident, dmaskT],
                          axis=1).astype(ml_dtypes.bfloat16)


def _consts_f32():
    r = np.arange(T)
    tri = r[None, :] - r[:, None]                           # t - s
    trik = np.where(tri >= 0, -K1 * tri - LNC2INV, -10000.0).astype(np.float32)
    xdvec = np.full((T, 1), C2 * XD, dtype=np.float32)
    xdvec[0, 0] = 0.0                                       # x_{-1} = 0
    return np.ascontiguousarray(np.concatenate([trik, xdvec], axis=1))


_cache = {}


def _split_multiwait(nc, mybir):
    """This walrus build caps sync waits per instruction (1 for regular
    instructions, 2 for EventSemaphore). Tile attaches more (e.g. the
    kernel-tail Drain waits on every live semaphore). Hoist excess waits
    onto same-engine NOPs placed immediately before the instruction —
    engine queues are sequential, so semantics are preserved."""
    n = 0
    for f in nc.m.functions:
        for bb in f.blocks:
            out = []
            changed = False
            for ins in bb.instructions:
                si = ins.sync_info
                ow = list(si.on_wait) if si is not None else []
                cap = 2 if ins.opcode == "EventSemaphore" else 1
                if len(ow) > cap:
                    sem_waits = [w for w in ow if w.sync_type == "semaphore"]
                    other = [w for w in ow if w.sync_type != "semaphore"]
                    keep = max(cap - len(other), 0)
                    hoist = sem_waits[:len(sem_waits) - keep] if keep else sem_waits
                    kept = sem_waits[len(hoist):] + other
                    assert len(kept) <= cap, (len(kept), cap, ins.opcode)
                    changed = True
                    for w in hoist:
                        n += 1
                        nop = mybir.InstNoOp(
                            name=f"wsplit-{n}",
                            sync_info=mybir.SyncInfo(on_wait=[w], on_update=[]),
                            bass_nofuse=True,
                            engine=ins.engine,
                        )
                        nc.register_instruction(nop, overwrite=True)
                        out.append(nop)
                    si.on_wait = kept
                out.append(ins)
            if changed:
                bb.instructions = out
    return nc


def _build():
    import concourse.bass as bass
    import concourse.mybir as mybir
    import concourse.tile as tile

    f32 = mybir.dt.float32
    bf16 = mybir.dt.bfloat16
    AF = mybir.ActivationFunctionType
    ALU = mybir.AluOpType
    AX = mybir.AxisListType

    from concourse.vector_clock import ScopedClock

    class _TrimTailTC(tile.TileContext):
        # Drop the second kernel-tail all-engine barrier: it only orders
        # the semaphore resets against engine halt, and nothing executes
        # after it. The first barrier (before resets) is kept, so resets
        # still happen on a quiesced machine and re-execution stays safe.
        def _drain_and_barrier(self, tick_clock, wait_clock):
            drain_inst = self.nc.sync.drain()
            wait_clock.add_sem_waits(
                drain_inst.ins, ScopedClock({None: tick_clock.global_clock})
            )
            self.nc.all_engine_barrier()
            assert self.sems is not None
            popped = self.nc._tile_sem_poison_stack.pop()
            assert popped is self._sem_poison
            self.nc.clear_and_free_semaphores(
                list(self.sems.allocated().values())
            )

    nc = bass.Bass()

    d_blob = nc.dram_tensor("blob", [128, CB], bf16, kind="ExternalInput")
    d_cf32 = nc.dram_tensor("cf32", [128, T + 1], f32, kind="ExternalInput")
    d_out = nc.dram_tensor("out", [T, D], f32, kind="ExternalOutput")

    with _TrimTailTC(nc) as tc, ExitStack() as ctx:
        work = ctx.enter_context(tc.tile_pool(name="work", bufs=1))
        stats = ctx.enter_context(tc.tile_pool(name="stats", bufs=1))
        # PSUM slots are bank-granular: 8 banks of [128,512]xf32 total.
        # Tags: pu(2) u-phase, ch(3) xt/tpb/yc rotation, g(1) g->aps->vps,
        # cc(2) qc/qr/bc/keepalive rotation.
        p_ps = ctx.enter_context(tc.tile_pool(name="p_ps", bufs=1, space="PSUM"))

        # ---- DMAs first: HWDGE FIFO order per queue = need order. -------
        blob_sb = work.tile([128, CB], bf16)
        cf32_sb = work.tile([128, T + 1], f32)
        P0 = C_DX + 2048                   # consts+embT+emb+dxT j01
        P1 = C_DX + 4096                   # dxT j23
        nc.sync.dma_start(blob_sb[:, 0:P0], d_blob[:, 0:P0])
        nc.sync.dma_start(blob_sb[:, P0:P1], d_blob[:, P0:P1])
        nc.sync.dma_start(blob_sb[:, C_DY:C_ET], d_blob[:, C_DY:C_ET])
        nc.sync.dma_start(blob_sb[:, C_ET:CB], d_blob[:, C_ET:CB])
        nc.scalar.dma_start(cf32_sb[:], d_cf32[:])

        ustrict_sb = blob_sb[:, C_US:C_US + T]
        ident_sb = blob_sb[:, C_ID:C_ID + T]
        dmaskT_sb = blob_sb[:, C_DM:C_DM + T]
        embT_sb = blob_sb[:, C_EMBT:C_EMBT + KD * T]
        emb_sb = blob_sb[:, C_EMB:C_EMB + D]
        dxT_sb = blob_sb[:, C_DX:C_DX + KD * N]
        dyT_sb = blob_sb[:, C_DY:C_DY + KD * N]
        eT_sb = blob_sb[:, C_ET:C_ET + KN * D]
        trik_sb = cf32_sb[:, 0:T]
        xdvec_sb = cf32_sb[:, T:T + 1]

        # ---- ACT table preload: Ln+Exp share one set; everything else the
        # kernel uses (Relu/Copy/Identity) is a filler in every set.
        pre_sb = stats.tile([1, 1], f32)
        nc.vector.memset(pre_sb[:], 1.0)
        pre_o = stats.tile([1, 1], f32)
        nc.scalar.activation(pre_o[:], pre_sb[:], AF.Ln)
        nc.scalar.activation(pre_o[:], pre_sb[:], AF.Exp)

        # ---- PE warmup: random-data bf16 matmuls while inputs stream ----
        # (all-zero operands leave the HAM activity monitor cold)
        wu_sb = work.tile([128, 128], bf16)
        nc.vector.random(wu_sb[:])
        wu_ps = p_ps.tile([128, 512], f32, tag="pu", bufs=2)
        for _ in range(WARMUP_MMS):
            nc.tensor.matmul(wu_ps[:, 0:128], wu_sb[:], wu_sb[:], start=True,
                             stop=True)

        def keepalive(ap):
            # PE matmuls gated on a late chain value: hold the HAM clock
            # warm through serial non-PE stretches. ap must be f32 [T,1].
            ka = p_ps.tile([T, T], f32, tag="cc", bufs=2)
            nc.tensor.matmul(ka[0:1, :], ap, trik_sb[:], start=True, stop=True)
            nc.tensor.matmul(ka[0:1, :], ap, trik_sb[:], start=True, stop=True)

        def ln_stats(src, tagp, hold_pe=False, cinv=None, cinv2=None):
            """scale/nmr for LN over the free dim. rstd = exp(-0.5*ln(v+eps))
            keeps everything in the natural_log_exp table set. When the rows
            of src carry a known positive scale c (cinv=1/c, cinv2=1/c^2),
            the stats are corrected so eps applies to the TRUE variance —
            LN is only scale-invariant when v >> eps, which fails for the
            near-zero early rows of a*."""
            stat6 = stats.tile([T, 6], f32, tag=f"{tagp}_s6")
            nc.vector.bn_stats(stat6[:], src)
            mv = stats.tile([T, 2], f32, tag=f"{tagp}_mv")
            nc.vector.bn_aggr(mv[:], stat6[:])
            if hold_pe:
                keepalive(mv[:, 0:1])
            veps = stats.tile([T, 1], f32, tag=f"{tagp}_ve")
            if cinv2 is None:
                nc.vector.tensor_scalar_add(veps[:], mv[:, 1:2], LN_EPS)
            else:
                nc.vector.tensor_scalar(veps[:], mv[:, 1:2], cinv2[:], LN_EPS,
                                        op0=ALU.mult, op1=ALU.add)
            lv = stats.tile([T, 1], f32, tag=f"{tagp}_lv")
            nc.scalar.activation(lv[:], veps[:], AF.Ln)
            rstd = stats.tile([T, 1], f32, tag=f"{tagp}_rs")
            nc.scalar.activation(rstd[:], lv[:], AF.Exp, scale=-0.5)
            if hold_pe:
                keepalive(rstd[:])
            if cinv is not None:
                scl = stats.tile([T, 1], f32, tag=f"{tagp}_sc")
                nc.vector.tensor_mul(scl[:], rstd[:], cinv[:])
            else:
                scl = rstd
            nmr = stats.tile([T, 1], f32, tag=f"{tagp}_nr")
            nc.vector.scalar_tensor_tensor(nmr[:], mv[:, 0:1], -1.0, scl[:],
                                           op0=ALU.mult, op1=ALU.mult)
            return scl, nmr

        # ---- u = relu(emb @ Dx.T) (bf16), row sums --------------------
        u_sb = work.tile([T, N], bf16)
        su_part = stats.tile([T, NJ], f32)
        for j in range(NJ):
            ps = p_ps.tile([128, 512], f32, tag="pu", bufs=2)
            for k in range(KD):
                nc.tensor.matmul(
                    ps[:],
                    embT_sb[:, k * T:(k + 1) * T],
                    dxT_sb[:, j * 1024 + k * 512: j * 1024 + (k + 1) * 512],
                    start=(k == 0),
                    stop=(k == KD - 1),
                )
            nc.vector.tensor_scalar(
                u_sb[:, j * 512:(j + 1) * 512], ps[:], 0.0, 0.0,
                op0=ALU.max, op1=ALU.add,
                accum_out=su_part[:, j:j + 1],
            )

        # ---- C^T coefficient matrix ------------------------------------
        su = stats.tile([T, 1], f32)
        nc.vector.tensor_reduce(su[:], su_part[:], axis=AX.X, op=ALU.add)
        keepalive(su[:])
        q_sb = stats.tile([T, 1], bf16)
        nc.scalar.activation(q_sb[:], su[:], AF.Ln, scale=C2, bias=xdvec_sb)

        # ct[s,t] = exp(Q_{s-1} + trik[s,t]): the true C also carries a
        # exp(-Q_t) column factor, but that scales a* and v rows by a
        # positive per-row constant, which the downstream LayerNorms cancel
        # exactly. The exp(-Q_s) row factor of W is applied via wneg below.
        qc = p_ps.tile([T, T], f32, tag="cc", bufs=2)     # Q_{s-1} column
        nc.tensor.matmul(qc[:, 0:1], ustrict_sb, q_sb[:], start=True, stop=True)
        colsc = stats.tile([T, 1], f32)
        nc.vector.tensor_copy(colsc[:], qc[:, 0:1])
        keepalive(colsc[:])
        expo = work.tile([T, T], f32)
        nc.vector.tensor_scalar(expo[:], trik_sb[:], colsc[:], -80.0,
                                op0=ALU.add, op1=ALU.max)
        ct_sb = work.tile([T, T], bf16)               # C^T [s,t]
        nc.scalar.activation(ct_sb[:], expo[:], AF.Exp)
        qcol = stats.tile([T, 1], f32)
        nc.vector.tensor_add(qcol[:], colsc[:], q_sb[:])
        wneg = stats.tile([T, 1], f32)                # exp(-Q_s)
        nc.scalar.activation(wneg[:], qcol[:], AF.Exp, scale=-1.0)
        cinv2 = stats.tile([T, 1], f32)               # exp(-2Q_s)
        nc.scalar.activation(cinv2[:], qcol[:], AF.Exp, scale=-2.0)

        # ---- vn = LN(emb) (off critical path) --------------------------
        vn_sb = work.tile([T, D], bf16)
        r_vn, n_vn = ln_stats(emb_sb, "vn")
        nc.gpsimd.tensor_scalar(vn_sb[:], emb_sb, r_vn[:], n_vn[:],
                                op0=ALU.mult, op1=ALU.add)

        # ---- X^T chunks = u_c @ C^T; G = X X^T, interleaved ------------
        xt_sb = work.tile([128, N], bf16)
        g = p_ps.tile([T, T], f32, tag="g", bufs=1)

        def xt_mm(c):
            tp = p_ps.tile([128, T], f32, tag="ch", bufs=3)
            nc.tensor.matmul(tp[:], u_sb[:, c * T:(c + 1) * T], ct_sb[:],
                             start=True, stop=True)
            if c % 2 == 0:
                nc.vector.tensor_copy(xt_sb[:, c * T:(c + 1) * T], tp[:])
            else:
                nc.scalar.copy(xt_sb[:, c * T:(c + 1) * T], tp[:])

        for c in range(KN):
            xt_mm(c)
            if c >= 2:
                cg = c - 2
                nc.tensor.matmul(g[:], xt_sb[:, cg * T:(cg + 1) * T],
                                 xt_sb[:, cg * T:(cg + 1) * T],
                                 start=(cg == 0), stop=False)
        for cg in (KN - 2, KN - 1):
            nc.tensor.matmul(g[:], xt_sb[:, cg * T:(cg + 1) * T],
                             xt_sb[:, cg * T:(cg + 1) * T],
                             start=False, stop=(cg == KN - 1))

        # ---- a* = (G . dmask)^T @ vn, LN, transpose --------------------
        wt_sb = work.tile([T, T], bf16)
        nc.vector.scalar_tensor_tensor(wt_sb[:], g[:], wneg[:], dmaskT_sb,
                                       op0=ALU.mult, op1=ALU.mult)
        aps = p_ps.tile([T, D], f32, tag="g", bufs=1)
        nc.tensor.matmul(aps[:], wt_sb[:], vn_sb[:], start=True, stop=True)
        r_a, n_a = ln_stats(aps[:], "la", hold_pe=True, cinv=wneg,
                            cinv2=cinv2)
        lna_sb = work.tile([T, D], bf16)
        nc.scalar.activation(lna_sb[:, 0:T], aps[:, 0:T], AF.Identity,
                             scale=r_a[:], bias=n_a[:])
        nc.vector.tensor_scalar(lna_sb[:, T:D], aps[:, T:D], r_a[:], n_a[:],
                                op0=ALU.mult, op1=ALU.add)

        lnaT_sb = work.tile([128, KD * T], bf16)
        for k in range(KD):
            tp = p_ps.tile([128, T], bf16, tag="ch", bufs=3)
            nc.tensor.transpose(tp[:], lna_sb[:, k * T:(k + 1) * T], ident_sb)
            if k % 2 == 0:
                nc.vector.tensor_copy(lnaT_sb[:, k * T:(k + 1) * T], tp[:])
            else:
                nc.scalar.copy(lnaT_sb[:, k * T:(k + 1) * T], tp[:])

        # ---- Ycore^T chunks -> Y^T = relu(.)*X^T -> v accumulation -----
        yt_sb = work.tile([128, N], bf16)
        vps = p_ps.tile([T, D], f32, tag="g", bufs=1)

        def yc_mm(c):
            yc = p_ps.tile([128, T], f32, tag="ch", bufs=3)
            for k in range(KD):
                nc.tensor.matmul(
                    yc[:],
                    dyT_sb[:, c * 256 + k * 128: c * 256 + (k + 1) * 128],
                    lnaT_sb[:, k * T:(k + 1) * T],
                    start=(k == 0), stop=(k == KD - 1),
                )
            if c % 2 == 0:
                nc.vector.scalar_tensor_tensor(
                    yt_sb[:, c * T:(c + 1) * T], yc[:], 0.0,
                    xt_sb[:, c * T:(c + 1) * T], op0=ALU.max, op1=ALU.mult,
                )
            else:
                ycr = work.tile([128, T], bf16, tag="ycr", bufs=2)
                nc.scalar.activation(ycr[:], yc[:], AF.Relu)
                nc.gpsimd.tensor_mul(yt_sb[:, c * T:(c + 1) * T], ycr[:],
                                     xt_sb[:, c * T:(c + 1) * T])

        for c in range(KN):
            yc_mm(c)
            if c >= 2:
                cv = c - 2
                nc.tensor.matmul(vps[:], yt_sb[:, cv * T:(cv + 1) * T],
                                 eT_sb[:, cv * D:(cv + 1) * D],
                                 start=(cv == 0), stop=False)
        for cv in (KN - 2, KN - 1):
            nc.tensor.matmul(vps[:], yt_sb[:, cv * T:(cv + 1) * T],
                             eT_sb[:, cv * D:(cv + 1) * D],
                             start=False, stop=(cv == KN - 1))

        # ---- v* = LN(vps) -> out ---------------------------------------
        r_v, n_v = ln_stats(vps[:], "vs", cinv=wneg, cinv2=cinv2)
        vstar_sb = work.tile([T, D], f32)
        nc.scalar.activation(vstar_sb[:, 0:T], vps[:, 0:T], AF.Identity,
                             scale=r_v[:], bias=n_v[:])
        nc.vector.tensor_scalar(vstar_sb[:, T:D], vps[:, T:D], r_v[:], n_v[:],
                                op0=ALU.mult, op1=ALU.add)
        nc.sync.dma_start(d_out[:], vstar_sb[:])

    return _split_multiwait(nc, mybir)


def _numpy_fallback(embeddings, E, Dx, Dy, x_state, rho_state):
    # General-path reference (only used if initial states are nonzero).
    def ln(x):
        m = x.mean(-1, keepdims=True)
        v = ((x - m) ** 2).mean(-1, keepdims=True)
        return (x - m) / np.sqrt(v + LN_EPS)

    x_s = x_state.astype(np.float32).copy()
    rho = rho_state.astype(np.float32).copy()
    outs = np.zeros((B, T, D), dtype=np.float32)
    for t in range(T):
        v_prev = embeddings[:, t, :]
        x_upd = np.maximum(v_prev @ Dx.T, 0.0)
        x_t = XD * x_s + x_upd
        x_t = x_t / np.maximum(np.abs(x_t).sum(-1, keepdims=True), 1e-12)
        a_star = np.einsum("bdn,bn->bd", rho, x_t)
        y_core = ln(a_star) @ Dy.T
        y_t = np.maximum(y_core, 0.0) * np.maximum(x_t, 0.0)
        outs[:, t, :] = ln(y_t @ E.T)
        vn = ln(v_prev)
        rho = UD * rho + np.einsum("bd,bn->bdn", vn, x_t)
        x_s = x_t
    return outs


def kernel(embeddings, E, Dx, Dy, x_state, rho_state):
    import ml_dtypes

    embeddings = np.ascontiguousarray(embeddings, dtype=np.float32)
    E = np.ascontiguousarray(E, dtype=np.float32)
    Dx = np.ascontiguousarray(Dx, dtype=np.float32)
    Dy = np.ascontiguousarray(Dy, dtype=np.float32)

    if np.any(x_state) or np.any(rho_state):
        return _numpy_fallback(embeddings, E, Dx, Dy,
                               np.asarray(x_state, np.float32),
                               np.asarray(rho_state, np.float32))

    from concourse.bass_utils import run_bass_kernel_spmd

    if "nc" not in _cache:
        _cache["nc"] = _build()
    nc = _cache["nc"]

    bf = ml_dtypes.bfloat16
    consts_bf = _consts_bf16()
    cf32 = _consts_f32()
    # SBUF-layout packing: row p holds that partition's contiguous span.
    dxT = np.ascontiguousarray(
        Dx.T.reshape(KD, 128, NJ, 512).transpose(1, 2, 0, 3).reshape(128, KD * N)
    ).astype(bf)
    dyT = np.ascontiguousarray(
        Dy.T.reshape(KD, 128, KN, 128).transpose(1, 2, 0, 3).reshape(128, KD * N)
    ).astype(bf)
    eT = np.ascontiguousarray(
        E.T.reshape(KN, 128, D).transpose(1, 0, 2).reshape(128, KN * D)
    ).astype(bf)

    in_maps = []
    for b in range(B):
        emb_b = embeddings[b]
        embT_b = np.ascontiguousarray(
            emb_b.T.reshape(KD, 128, T).transpose(1, 0, 2).reshape(128, KD * T)
        ).astype(bf)
        blob = np.concatenate(
            [consts_bf, embT_b, emb_b.astype(bf), dxT, dyT, eT], axis=1)
        assert blob.shape == (128, CB), blob.shape
        in_maps.append({"blob": np.ascontiguousarray(blob), "cf32": cf32})

    res = run_bass_kernel_spmd(nc, in_maps, list(range(B)))
    _cache["last_results"] = res
    return np.stack([res.results[i]["out"] for i in range(B)])


# revision 9
# speedup vs baseline: 1.4165x; 1.0887x over previous
"""Trainium2 Bass kernel for the BDH-style recurrent block.

Strategy: data-parallel over B (8 batches -> 8 NeuronCores, no collectives).
The T=128-step scan is de-sequentialized into dense matmuls per core:

  u_t = relu(emb_t @ Dx.T)                                  (T,N)
  x_t = (XD*x_{t-1} + u_t)/s_t  with s_t = XD + sum(u_t)    (L1 norm; x>=0)
      => X = C @ u, C[t,s] = (1/c_s) exp(A_t - A_s), A_t = cumsum log(XD/c_r)
  a*_t = rho_{t-1} @ x_t = ((DecayMask . X X^T) @ ln(emb))_t   (rho_0 = 0)
  y_t  = relu(ln(a*_t) @ Dy.T) * x_t                        (x_t >= 0)
  v*_t = ln(y_t @ E.T)

All matmuls run in bf16 (1 col/cycle at any free dim, vs f32r's 4 cyc/row
below 256 free): X^T and Ycore^T are produced directly in n-major chunks
(lhsT = u chunk / DyT chunk), so no PE transposes of X/Y are needed.
LayerNorm rstd uses exp(-0.5*ln(v+eps)) so the only ACT table set ever
needed is natural_log_exp_and_others -> one table load, at kernel start.
Inputs arrive as one packed bf16 blob + a small f32 const tensor, posted
in need-order across the Sync and Scalar HWDGE queues.
"""

import math
from contextlib import ExitStack

import numpy as np

N = 2048
D = 256
B = 8
T = 128
XD = 0.97
UD = 0.97
LN_EPS = 1e-5

# log-domain recentring: E[sum relu(N(0,1)) over 2048] + XD ~ 818.9
LNC2INV = 6.7065
C2 = math.exp(-LNC2INV)
K1 = LNC2INV - math.log(XD)

KD = D // 128    # 2
KN = N // 128    # 16
NJ = N // 512    # 4
WARMUP_MMS = 14  # 256-col bf16 MMs while the first DMA piece streams

# bf16 blob column layout (per 128-partition row)
C_US = 0                 # ustrict [s< t]
C_ID = C_US + T          # identity
C_DM = C_ID + T          # dmaskT  UD^(t-1-s)
C_EMBT = C_DM + T        # embT (KD*T)
C_EMB = C_EMBT + KD * T  # emb  (D)
C_DX = C_EMB + D         # dxT packed [j(4), k(2), 512]
C_DY = C_DX + KD * N     # dyT packed [c(16), k(2), 128]
C_ET = C_DY + KD * N     # eT  packed [c(16), 256]
CB = C_ET + KN * D


def _consts_bf16():
    import ml_dtypes
    r = np.arange(T)
    ustrict = (r[:, None] < r[None, :]).astype(np.float32)
    ident = np.eye(T, dtype=np.float32)
    pw = r[:, None] - 1 - r[None, :]                        # [t,s] t-1-s
    dmask = np.where(pw >= 0, UD ** np.maximum(pw, 0), 0.0).astype(np.float32)
    dmaskT = np.ascontiguousarray(dmask.T)                  # [s,t]
    return np.concatenate([ustrict, ident, dmaskT],
                          axis=1).astype(ml_dtypes.bfloat16)


def _consts_f32():
    r = np.arange(T)
    tri = r[None, :] - r[:, None]                           # t - s
    trik = np.where(tri >= 0, -K1 * tri - LNC2INV, -10000.0).astype(np.float32)
    xdvec = np.full((T, 1), C2 * XD, dtype=np.float32)
    xdvec[0, 0] = 0.0                                       # x_{-1} = 0
    return np.ascontiguousarray(np.concatenate([trik, xdvec], axis=1))


_cache = {}


def _split_multiwait(nc, mybir):
    """This walrus build caps sync waits per instruction (1 for regular
    instructions, 2 for EventSemaphore). Tile attaches more (e.g. the
    kernel-tail Drain waits on every live semaphore). Hoist excess waits
    onto same-engine NOPs placed immediately before the instruction —
    engine queues are sequential, so semantics are preserved."""
    n = 0
    for f in nc.m.functions:
        for bb in f.blocks:
            out = []
            changed = False
            for ins in bb.instructions:
                si = ins.sync_info
                ow = list(si.on_wait) if si is not None else []
                cap = 2 if ins.opcode == "EventSemaphore" else 1
                if len(ow) > cap:
                    sem_waits = [w for w in ow if w.sync_type == "semaphore"]
                    other = [w for w in ow if w.sync_type != "semaphore"]
                    keep = max(cap - len(other), 0)
                    hoist = sem_waits[:len(sem_waits) - keep] if keep else sem_waits
                    kept = sem_waits[len(hoist):] + other
                    assert len(kept) <= cap, (len(kept), cap, ins.opcode)
                    changed = True
                    for w in hoist:
                        n += 1
                        nop = mybir.InstNoOp(
                            name=f"wsplit-{n}",
                            sync_info=mybir.SyncInfo(on_wait=[w], on_update=[]),
                            bass_nofuse=True,
                            engine=ins.engine,
                        )
                        nc.register_instruction(nop, overwrite=True)
                        out.append(nop)
                    si.on_wait = kept
                out.append(ins)
            if changed:
                bb.instructions = out
    return nc


def _build():
    import concourse.bass as bass
    import concourse.mybir as mybir
    import concourse.tile as tile

    f32 = mybir.dt.float32
    bf16 = mybir.dt.bfloat16
    AF = mybir.ActivationFunctionType
    ALU = mybir.AluOpType
    AX = mybir.AxisListType

    from concourse.vector_clock import ScopedClock

    class _TrimTailTC(tile.TileContext):
        # Drop the second kernel-tail all-engine barrier: it only orders
        # the semaphore resets against engine halt, and nothing executes
        # after it. The first barrier (before resets) is kept, so resets
        # still happen on a quiesced machine and re-execution stays safe.
        def _drain_and_barrier(self, tick_clock, wait_clock):
            drain_inst = self.nc.sync.drain()
            wait_clock.add_sem_waits(
                drain_inst.ins, ScopedClock({None: tick_clock.global_clock})
            )
            self.nc.all_engine_barrier()
            assert self.sems is not None
            popped = self.nc._tile_sem_poison_stack.pop()
            assert popped is self._sem_poison
            self.nc.clear_and_free_semaphores(
                list(self.sems.allocated().values())
            )

    nc = bass.Bass()

    d_blob = nc.dram_tensor("blob", [128, CB], bf16, kind="ExternalInput")
    d_cf32 = nc.dram_tensor("cf32", [128, T + 1], f32, kind="ExternalInput")
    d_out = nc.dram_tensor("out", [T, D], f32, kind="ExternalOutput")

    with _TrimTailTC(nc) as tc, ExitStack() as ctx:
        work = ctx.enter_context(tc.tile_pool(name="work", bufs=1))
        stats = ctx.enter_context(tc.tile_pool(name="stats", bufs=1))
        # PSUM slots are bank-granular: 8 banks of [128,512]xf32 total.
        # Tags: pu(2) u-phase, ch(3) xt/tpb/yc rotation, g(1) g->aps->vps,
        # cc(2) qc/qr/bc/keepalive rotation.
        p_ps = ctx.enter_context(tc.tile_pool(name="p_ps", bufs=1, space="PSUM"))

        # ---- DMAs first: HWDGE FIFO order per queue = need order. -------
        blob_sb = work.tile([128, CB], bf16)
        cf32_sb = work.tile([128, T + 1], f32)
        # Per-j dxT pieces: the DMA completion semaphore fires ~1.5us after
        # the last byte (write-receipt round trip); small pieces pipeline
        # that lag so u j0 can start as early as possible.
        cuts = [C_DX + 1024 * j for j in range(NJ + 1)] + [C_ET, CB]
        cuts[0] = 0
        for a, b in zip(cuts[:-1], cuts[1:]):
            nc.sync.dma_start(blob_sb[:, a:b], d_blob[:, a:b])
        nc.scalar.dma_start(cf32_sb[:], d_cf32[:])

        ustrict_sb = blob_sb[:, C_US:C_US + T]
        ident_sb = blob_sb[:, C_ID:C_ID + T]
        dmaskT_sb = blob_sb[:, C_DM:C_DM + T]
        embT_sb = blob_sb[:, C_EMBT:C_EMBT + KD * T]
        emb_sb = blob_sb[:, C_EMB:C_EMB + D]
        dxT_sb = blob_sb[:, C_DX:C_DX + KD * N]
        dyT_sb = blob_sb[:, C_DY:C_DY + KD * N]
        eT_sb = blob_sb[:, C_ET:C_ET + KN * D]
        trik_sb = cf32_sb[:, 0:T]
        xdvec_sb = cf32_sb[:, T:T + 1]

        # ---- ACT table preload: Ln+Exp share one set; everything else the
        # kernel uses (Relu/Copy/Identity) is a filler in every set.
        pre_sb = stats.tile([1, 1], f32)
        nc.vector.memset(pre_sb[:], 1.0)
        pre_o = stats.tile([1, 1], f32)
        nc.scalar.activation(pre_o[:], pre_sb[:], AF.Ln)
        nc.scalar.activation(pre_o[:], pre_sb[:], AF.Exp)

        # ---- PE warmup: random-data bf16 matmuls while inputs stream ----
        # (all-zero operands leave the HAM activity monitor cold)
        wu_sb = work.tile([128, 256], bf16)
        nc.vector.random(wu_sb[:])
        wu_ps = p_ps.tile([128, 512], f32, tag="pu", bufs=2)
        for _ in range(WARMUP_MMS):
            nc.tensor.matmul(wu_ps[:, 0:256], wu_sb[:, 0:128], wu_sb[:],
                             start=True, stop=True)

        def keepalive(ap):
            # PE matmuls gated on a late chain value: hold the HAM clock
            # warm through serial non-PE stretches. ap must be f32 [T,1].
            ka = p_ps.tile([T, T], f32, tag="cc", bufs=2)
            nc.tensor.matmul(ka[0:1, :], ap, trik_sb[:], start=True, stop=True)
            nc.tensor.matmul(ka[0:1, :], ap, trik_sb[:], start=True, stop=True)

        def ln_stats(src, tagp, hold_pe=False, cinv=None, cinv2=None):
            """scale/nmr for LN over the free dim. rstd = exp(-0.5*ln(v+eps))
            keeps everything in the natural_log_exp table set. When the rows
            of src carry a known positive scale c (cinv=1/c, cinv2=1/c^2),
            the stats are corrected so eps applies to the TRUE variance —
            LN is only scale-invariant when v >> eps, which fails for the
            near-zero early rows of a*."""
            stat6 = stats.tile([T, 6], f32, tag=f"{tagp}_s6")
            nc.vector.bn_stats(stat6[:], src)
            mv = stats.tile([T, 2], f32, tag=f"{tagp}_mv")
            nc.vector.bn_aggr(mv[:], stat6[:])
            if hold_pe:
                keepalive(mv[:, 0:1])
            veps = stats.tile([T, 1], f32, tag=f"{tagp}_ve")
            if cinv2 is None:
                nc.vector.tensor_scalar_add(veps[:], mv[:, 1:2], LN_EPS)
            else:
                nc.vector.tensor_scalar(veps[:], mv[:, 1:2], cinv2[:], LN_EPS,
                                        op0=ALU.mult, op1=ALU.add)
            lv = stats.tile([T, 1], f32, tag=f"{tagp}_lv")
            nc.scalar.activation(lv[:], veps[:], AF.Ln)
            rstd = stats.tile([T, 1], f32, tag=f"{tagp}_rs")
            nc.scalar.activation(rstd[:], lv[:], AF.Exp, scale=-0.5)
            if hold_pe:
                keepalive(rstd[:])
            if cinv is not None:
                scl = stats.tile([T, 1], f32, tag=f"{tagp}_sc")
                nc.vector.tensor_mul(scl[:], rstd[:], cinv[:])
            else:
                scl = rstd
            nmr = stats.tile([T, 1], f32, tag=f"{tagp}_nr")
            nc.vector.scalar_tensor_tensor(nmr[:], mv[:, 0:1], -1.0, scl[:],
                                           op0=ALU.mult, op1=ALU.mult)
            return scl, nmr

        # ---- u = relu(emb @ Dx.T) (bf16), row sums --------------------
        u_sb = work.tile([T, N], bf16)
        su_part = stats.tile([T, NJ], f32)
        for j in range(NJ):
            ps = p_ps.tile([128, 512], f32, tag="pu", bufs=2)
            for k in range(KD):
                nc.tensor.matmul(
                    ps[:],
                    embT_sb[:, k * T:(k + 1) * T],
                    dxT_sb[:, j * 1024 + k * 512: j * 1024 + (k + 1) * 512],
                    start=(k == 0),
                    stop=(k == KD - 1),
                )
            nc.vector.tensor_scalar(
                u_sb[:, j * 512:(j + 1) * 512], ps[:], 0.0, 0.0,
                op0=ALU.max, op1=ALU.add,
                accum_out=su_part[:, j:j + 1],
            )

        # ---- C^T coefficient matrix ------------------------------------
        su = stats.tile([T, 1], f32)
        nc.vector.tensor_reduce(su[:], su_part[:], axis=AX.X, op=ALU.add)
        keepalive(su[:])
        q_sb = stats.tile([T, 1], bf16)
        nc.scalar.activation(q_sb[:], su[:], AF.Ln, scale=C2, bias=xdvec_sb)

        # ct[s,t] = exp(Q_{s-1} + trik[s,t]): the true C also carries a
        # exp(-Q_t) column factor, but that scales a* and v rows by a
        # positive per-row constant, which the downstream LayerNorms cancel
        # exactly. The exp(-Q_s) row factor of W is applied via wneg below.
        qc = p_ps.tile([T, T], f32, tag="cc", bufs=2)     # Q_{s-1} column
        nc.tensor.matmul(qc[:, 0:1], ustrict_sb, q_sb[:], start=True, stop=True)
        colsc = stats.tile([T, 1], f32)
        nc.vector.tensor_copy(colsc[:], qc[:, 0:1])
        keepalive(colsc[:])
        expo = work.tile([T, T], f32)
        nc.vector.tensor_scalar(expo[:], trik_sb[:], colsc[:], -80.0,
                                op0=ALU.add, op1=ALU.max)
        ct_sb = work.tile([T, T], bf16)               # C^T [s,t]
        nc.scalar.activation(ct_sb[:], expo[:], AF.Exp)
        qcol = stats.tile([T, 1], f32)
        nc.vector.tensor_add(qcol[:], colsc[:], q_sb[:])
        wneg = stats.tile([T, 1], f32)                # exp(-Q_s)
        nc.scalar.activation(wneg[:], qcol[:], AF.Exp, scale=-1.0)
        cinv2 = stats.tile([T, 1], f32)               # exp(-2Q_s)
        nc.scalar.activation(cinv2[:], qcol[:], AF.Exp, scale=-2.0)

        # ---- vn = LN(emb) (off critical path) --------------------------
        vn_sb = work.tile([T, D], bf16)
        r_vn, n_vn = ln_stats(emb_sb, "vn")
        nc.gpsimd.tensor_scalar(vn_sb[:], emb_sb, r_vn[:], n_vn[:],
                                op0=ALU.mult, op1=ALU.add)

        # ---- X^T chunks = u_c @ C^T; G = X X^T, interleaved ------------
        xt_sb = work.tile([128, N], bf16)
        g = p_ps.tile([T, T], f32, tag="g", bufs=1)

        def xt_mm(c):
            tp = p_ps.tile([128, T], f32, tag="ch", bufs=3)
            nc.tensor.matmul(tp[:], u_sb[:, c * T:(c + 1) * T], ct_sb[:],
                             start=True, stop=True)
            if c % 2 == 0:
                nc.vector.tensor_copy(xt_sb[:, c * T:(c + 1) * T], tp[:])
            else:
                nc.scalar.copy(xt_sb[:, c * T:(c + 1) * T], tp[:])

        for c in range(KN):
            xt_mm(c)
            if c >= 2:
                cg = c - 2
                nc.tensor.matmul(g[:], xt_sb[:, cg * T:(cg + 1) * T],
                                 xt_sb[:, cg * T:(cg + 1) * T],
                                 start=(cg == 0), stop=False)
        for cg in (KN - 2, KN - 1):
            nc.tensor.matmul(g[:], xt_sb[:, cg * T:(cg + 1) * T],
                             xt_sb[:, cg * T:(cg + 1) * T],
                             start=False, stop=(cg == KN - 1))

        # ---- a* = (G . dmask)^T @ vn, LN, transpose --------------------
        wt_sb = work.tile([T, T], bf16)
        nc.vector.scalar_tensor_tensor(wt_sb[:], g[:], wneg[:], dmaskT_sb,
                                       op0=ALU.mult, op1=ALU.mult)
        aps = p_ps.tile([T, D], f32, tag="g", bufs=1)
        nc.tensor.matmul(aps[:], wt_sb[:], vn_sb[:], start=True, stop=True)
        r_a, n_a = ln_stats(aps[:], "la", hold_pe=True, cinv=wneg,
                            cinv2=cinv2)
        lna_sb = work.tile([T, D], bf16)
        nc.scalar.activation(lna_sb[:, 0:T], aps[:, 0:T], AF.Identity,
                             scale=r_a[:], bias=n_a[:])
        nc.vector.tensor_scalar(lna_sb[:, T:D], aps[:, T:D], r_a[:], n_a[:],
                                op0=ALU.mult, op1=ALU.add)

        lnaT_sb = work.tile([128, KD * T], bf16)
        for k in range(KD):
            tp = p_ps.tile([128, T], bf16, tag="ch", bufs=3)
            nc.tensor.transpose(tp[:], lna_sb[:, k * T:(k + 1) * T], ident_sb)
            if k % 2 == 0:
                nc.vector.tensor_copy(lnaT_sb[:, k * T:(k + 1) * T], tp[:])
            else:
                nc.scalar.copy(lnaT_sb[:, k * T:(k + 1) * T], tp[:])

        # ---- Ycore^T chunks -> Y^T = relu(.)*X^T -> v accumulation -----
        yt_sb = work.tile([128, N], bf16)
        vps = p_ps.tile([T, D], f32, tag="g", bufs=1)

        def yc_mm(c):
            yc = p_ps.tile([128, T], f32, tag="ch", bufs=3)
            for k in range(KD):
                nc.tensor.matmul(
                    yc[:],
                    dyT_sb[:, c * 256 + k * 128: c * 256 + (k + 1) * 128],
                    lnaT_sb[:, k * T:(k + 1) * T],
                    start=(k == 0), stop=(k == KD - 1),
                )
            if c % 2 == 0:
                nc.vector.scalar_tensor_tensor(
                    yt_sb[:, c * T:(c + 1) * T], yc[:], 0.0,
                    xt_sb[:, c * T:(c + 1) * T], op0=ALU.max, op1=ALU.mult,
                )
            else:
                ycr = work.tile([128, T], bf16, tag="ycr", bufs=2)
                nc.scalar.activation(ycr[:], yc[:], AF.Relu)
                nc.gpsimd.tensor_mul(yt_sb[:, c * T:(c + 1) * T], ycr[:],
                                     xt_sb[:, c * T:(c + 1) * T])

        for c in range(KN):
            yc_mm(c)
            if c >= 2:
                cv = c - 2
                nc.tensor.matmul(vps[:], yt_sb[:, cv * T:(cv + 1) * T],
                                 eT_sb[:, cv * D:(cv + 1) * D],
                                 start=(cv == 0), stop=False)
        for cv in (KN - 2, KN - 1):
            nc.tensor.matmul(vps[:], yt_sb[:, cv * T:(cv + 1) * T],
                             eT_sb[:, cv * D:(cv + 1) * D],
                             start=False, stop=(cv == KN - 1))

        # ---- v* = LN(vps) -> out ---------------------------------------
        r_v, n_v = ln_stats(vps[:], "vs", cinv=wneg, cinv2=cinv2)
        vstar_sb = work.tile([T, D], f32)
        nc.scalar.activation(vstar_sb[:, 0:T], vps[:, 0:T], AF.Identity,
                             scale=r_v[:], bias=n_v[:])
        nc.vector.tensor_scalar(vstar_sb[:, T:D], vps[:, T:D], r_v[:], n_v[:],
                                op0=ALU.mult, op1=ALU.add)
        nc.sync.dma_start(d_out[:], vstar_sb[:])

    return _split_multiwait(nc, mybir)


def _numpy_fallback(embeddings, E, Dx, Dy, x_state, rho_state):
    # General-path reference (only used if initial states are nonzero).
    def ln(x):
        m = x.mean(-1, keepdims=True)
        v = ((x - m) ** 2).mean(-1, keepdims=True)
        return (x - m) / np.sqrt(v + LN_EPS)

    x_s = x_state.astype(np.float32).copy()
    rho = rho_state.astype(np.float32).copy()
    outs = np.zeros((B, T, D), dtype=np.float32)
    for t in range(T):
        v_prev = embeddings[:, t, :]
        x_upd = np.maximum(v_prev @ Dx.T, 0.0)
        x_t = XD * x_s + x_upd
        x_t = x_t / np.maximum(np.abs(x_t).sum(-1, keepdims=True), 1e-12)
        a_star = np.einsum("bdn,bn->bd", rho, x_t)
        y_core = ln(a_star) @ Dy.T
        y_t = np.maximum(y_core, 0.0) * np.maximum(x_t, 0.0)
        outs[:, t, :] = ln(y_t @ E.T)
        vn = ln(v_prev)
        rho = UD * rho + np.einsum("bd,bn->bdn", vn, x_t)
        x_s = x_t
    return outs


def kernel(embeddings, E, Dx, Dy, x_state, rho_state):
    import ml_dtypes

    embeddings = np.ascontiguousarray(embeddings, dtype=np.float32)
    E = np.ascontiguousarray(E, dtype=np.float32)
    Dx = np.ascontiguousarray(Dx, dtype=np.float32)
    Dy = np.ascontiguousarray(Dy, dtype=np.float32)

    if np.any(x_state) or np.any(rho_state):
        return _numpy_fallback(embeddings, E, Dx, Dy,
                               np.asarray(x_state, np.float32),
                               np.asarray(rho_state, np.float32))

    from concourse.bass_utils import run_bass_kernel_spmd

    if "nc" not in _cache:
        _cache["nc"] = _build()
    nc = _cache["nc"]

    bf = ml_dtypes.bfloat16
    consts_bf = _consts_bf16()
    cf32 = _consts_f32()
    # SBUF-layout packing: row p holds that partition's contiguous span.
    dxT = np.ascontiguousarray(
        Dx.T.reshape(KD, 128, NJ, 512).transpose(1, 2, 0, 3).reshape(128, KD * N)
    ).astype(bf)
    dyT = np.ascontiguousarray(
        Dy.T.reshape(KD, 128, KN, 128).transpose(1, 2, 0, 3).reshape(128, KD * N)
    ).astype(bf)
    eT = np.ascontiguousarray(
        E.T.reshape(KN, 128, D).transpose(1, 0, 2).reshape(128, KN * D)
    ).astype(bf)

    in_maps = []
    for b in range(B):
        emb_b = embeddings[b]
        embT_b = np.ascontiguousarray(
            emb_b.T.reshape(KD, 128, T).transpose(1, 0, 2).reshape(128, KD * T)
        ).astype(bf)
        blob = np.concatenate(
            [consts_bf, embT_b, emb_b.astype(bf), dxT, dyT, eT], axis=1)
        assert blob.shape == (128, CB), blob.shape
        in_maps.append({"blob": np.ascontiguousarray(blob), "cf32": cf32})

    res = run_bass_kernel_spmd(nc, in_maps, list(range(B)))
    _cache["last_results"] = res
    return np.stack([res.results[i]["out"] for i in range(B)])


# revision 10
# speedup vs baseline: 1.4176x; 1.0008x over previous
"""Trainium2 Bass kernel for the BDH-style recurrent block.

Strategy: data-parallel over B (8 batches -> 8 NeuronCores, no collectives).
The T=128-step scan is de-sequentialized into dense matmuls per core:

  u_t = relu(emb_t @ Dx.T)                                  (T,N)
  x_t = (XD*x_{t-1} + u_t)/s_t  with s_t = XD + sum(u_t)    (L1 norm; x>=0)
      => X = C @ u, C[t,s] = (1/c_s) exp(A_t - A_s), A_t = cumsum log(XD/c_r)
  a*_t = rho_{t-1} @ x_t = ((DecayMask . X X^T) @ ln(emb))_t   (rho_0 = 0)
  y_t  = relu(ln(a*_t) @ Dy.T) * x_t                        (x_t >= 0)
  v*_t = ln(y_t @ E.T)

All matmuls run in bf16 (1 col/cycle at any free dim, vs f32r's 4 cyc/row
below 256 free): X^T and Ycore^T are produced directly in n-major chunks
(lhsT = u chunk / DyT chunk), so no PE transposes of X/Y are needed.
LayerNorm rstd uses exp(-0.5*ln(v+eps)) so the only ACT table set ever
needed is natural_log_exp_and_others -> one table load, at kernel start.
Inputs arrive as one packed bf16 blob + a small f32 const tensor, posted
in need-order across the Sync and Scalar HWDGE queues.
"""

import math
from contextlib import ExitStack

import numpy as np

N = 2048
D = 256
B = 8
T = 128
XD = 0.97
UD = 0.97
LN_EPS = 1e-5

# log-domain recentring: E[sum relu(N(0,1)) over 2048] + XD ~ 818.9
LNC2INV = 6.7065
C2 = math.exp(-LNC2INV)
K1 = LNC2INV - math.log(XD)

KD = D // 128    # 2
KN = N // 128    # 16
NJ = N // 512    # 4
WARMUP_MMS = 17  # 256-col bf16 MMs while the first DMA piece streams

# bf16 blob column layout (per 128-partition row)
C_US = 0                 # ustrict [s< t]
C_ID = C_US + T          # identity
C_DM = C_ID + T          # dmaskT  UD^(t-1-s)
C_EMBT = C_DM + T        # embT (KD*T)
C_EMB = C_EMBT + KD * T  # emb  (D)
C_DX = C_EMB + D         # dxT packed [j(4), k(2), 512]
C_DY = C_DX + KD * N     # dyT packed [c(16), k(2), 128]
C_ET = C_DY + KD * N     # eT  packed [c(16), 256]
CB = C_ET + KN * D


def _consts_bf16():
    import ml_dtypes
    r = np.arange(T)
    ustrict = (r[:, None] < r[None, :]).astype(np.float32)
    ident = np.eye(T, dtype=np.float32)
    pw = r[:, None] - 1 - r[None, :]                        # [t,s] t-1-s
    dmask = np.where(pw >= 0, UD ** np.maximum(pw, 0), 0.0).astype(np.float32)
    dmaskT = np.ascontiguousarray(dmask.T)                  # [s,t]
    return np.concatenate([ustrict, ident, dmaskT],
                          axis=1).astype(ml_dtypes.bfloat16)


def _consts_f32():
    r = np.arange(T)
    tri = r[None, :] - r[:, None]                           # t - s
    trik = np.where(tri >= 0, -K1 * tri - LNC2INV, -10000.0).astype(np.float32)
    xdvec = np.full((T, 1), C2 * XD, dtype=np.float32)
    xdvec[0, 0] = 0.0                                       # x_{-1} = 0
    zero = np.zeros((T, 1), dtype=np.float32)
    return np.ascontiguousarray(np.concatenate([trik, xdvec, zero], axis=1))


_cache = {}


def _split_multiwait(nc, mybir):
    """This walrus build caps sync waits per instruction (1 for regular
    instructions, 2 for EventSemaphore). Tile attaches more (e.g. the
    kernel-tail Drain waits on every live semaphore). Hoist excess waits
    onto same-engine NOPs placed immediately before the instruction —
    engine queues are sequential, so semantics are preserved."""
    n = 0
    for f in nc.m.functions:
        for bb in f.blocks:
            out = []
            changed = False
            for ins in bb.instructions:
                si = ins.sync_info
                ow = list(si.on_wait) if si is not None else []
                cap = 2 if ins.opcode == "EventSemaphore" else 1
                if len(ow) > cap:
                    sem_waits = [w for w in ow if w.sync_type == "semaphore"]
                    other = [w for w in ow if w.sync_type != "semaphore"]
                    keep = max(cap - len(other), 0)
                    hoist = sem_waits[:len(sem_waits) - keep] if keep else sem_waits
                    kept = sem_waits[len(hoist):] + other
                    assert len(kept) <= cap, (len(kept), cap, ins.opcode)
                    changed = True
                    for w in hoist:
                        n += 1
                        nop = mybir.InstNoOp(
                            name=f"wsplit-{n}",
                            sync_info=mybir.SyncInfo(on_wait=[w], on_update=[]),
                            bass_nofuse=True,
                            engine=ins.engine,
                        )
                        nc.register_instruction(nop, overwrite=True)
                        out.append(nop)
                    si.on_wait = kept
                out.append(ins)
            if changed:
                bb.instructions = out
    return nc


def _build():
    import concourse.bass as bass
    import concourse.mybir as mybir
    import concourse.tile as tile

    f32 = mybir.dt.float32
    bf16 = mybir.dt.bfloat16
    AF = mybir.ActivationFunctionType
    ALU = mybir.AluOpType
    AX = mybir.AxisListType

    from concourse.vector_clock import ScopedClock

    class _TrimTailTC(tile.TileContext):
        # Drop the second kernel-tail all-engine barrier: it only orders
        # the semaphore resets against engine halt, and nothing executes
        # after it. The first barrier (before resets) is kept, so resets
        # still happen on a quiesced machine and re-execution stays safe.
        def _drain_and_barrier(self, tick_clock, wait_clock):
            drain_inst = self.nc.sync.drain()
            wait_clock.add_sem_waits(
                drain_inst.ins, ScopedClock({None: tick_clock.global_clock})
            )
            self.nc.all_engine_barrier()
            assert self.sems is not None
            popped = self.nc._tile_sem_poison_stack.pop()
            assert popped is self._sem_poison
            self.nc.clear_and_free_semaphores(
                list(self.sems.allocated().values())
            )

    nc = bass.Bass()

    d_blob = nc.dram_tensor("blob", [128, CB], bf16, kind="ExternalInput")
    d_cf32 = nc.dram_tensor("cf32", [128, T + 2], f32, kind="ExternalInput")
    d_out = nc.dram_tensor("out", [T, D], f32, kind="ExternalOutput")

    with _TrimTailTC(nc) as tc, ExitStack() as ctx:
        work = ctx.enter_context(tc.tile_pool(name="work", bufs=1))
        stats = ctx.enter_context(tc.tile_pool(name="stats", bufs=1))
        # PSUM slots are bank-granular: 8 banks of [128,512]xf32 total.
        # Tags: pu(2) u-phase, ch(3) xt/tpb/yc rotation, g(1) g->aps->vps,
        # cc(2) qc/qr/bc/keepalive rotation.
        p_ps = ctx.enter_context(tc.tile_pool(name="p_ps", bufs=1, space="PSUM"))

        # ---- DMAs first: HWDGE FIFO order per queue = need order. -------
        blob_sb = work.tile([128, CB], bf16)
        cf32_sb = work.tile([128, T + 2], f32)
        # Per-j dxT pieces: the DMA completion semaphore fires ~1.5us after
        # the last byte (write-receipt round trip); small pieces pipeline
        # that lag so u j0 can start as early as possible.
        cuts = [C_DX + 1024 * j for j in range(NJ + 1)] + [C_ET, CB]
        cuts[0] = 0
        for a, b in zip(cuts[:-1], cuts[1:]):
            nc.sync.dma_start(blob_sb[:, a:b], d_blob[:, a:b])
        nc.scalar.dma_start(cf32_sb[:], d_cf32[:])

        ustrict_sb = blob_sb[:, C_US:C_US + T]
        ident_sb = blob_sb[:, C_ID:C_ID + T]
        dmaskT_sb = blob_sb[:, C_DM:C_DM + T]
        embT_sb = blob_sb[:, C_EMBT:C_EMBT + KD * T]
        emb_sb = blob_sb[:, C_EMB:C_EMB + D]
        dxT_sb = blob_sb[:, C_DX:C_DX + KD * N]
        dyT_sb = blob_sb[:, C_DY:C_DY + KD * N]
        eT_sb = blob_sb[:, C_ET:C_ET + KN * D]
        trik_sb = cf32_sb[:, 0:T]
        xdvec_sb = cf32_sb[:, T:T + 1]
        zero_sb = cf32_sb[:, T + 1:T + 2]

        # ---- ACT table preload: Ln+Exp share one set; everything else the
        # kernel uses (Relu/Copy/Identity) is a filler in every set.
        pre_o = stats.tile([1, 1], f32)
        nc.scalar.activation(pre_o[:], zero_sb[0:1, :], AF.Ln,
                             bias=zero_sb[0:1, :])
        nc.scalar.activation(pre_o[:], zero_sb[0:1, :], AF.Exp,
                             bias=zero_sb[0:1, :])

        # ---- PE warmup: random-data bf16 matmuls while inputs stream ----
        # (all-zero operands leave the HAM activity monitor cold)
        wu_t = nc.alloc_sbuf_tensor("wu_raw", [128, 256], bf16)
        wu_sb = wu_t.ap()
        wu_ps = p_ps.tile([128, 512], f32, tag="pu", bufs=2)
        for _ in range(WARMUP_MMS):
            nc.tensor.matmul(wu_ps[:, 0:256], wu_sb[:, 0:128], wu_sb[:],
                             start=True, stop=True)

        def keepalive(ap):
            # PE matmuls gated on a late chain value: hold the HAM clock
            # warm through serial non-PE stretches. ap must be f32 [T,1].
            ka = p_ps.tile([T, T], f32, tag="cc", bufs=2)
            nc.tensor.matmul(ka[0:1, :], ap, trik_sb[:], start=True, stop=True)
            nc.tensor.matmul(ka[0:1, :], ap, trik_sb[:], start=True, stop=True)

        def ln_stats(src, tagp, hold_pe=False, nq=None, cinv2=None):
            """scale/nmr for LN over the free dim. rstd = exp(-0.5*ln(v+eps))
            keeps everything in the natural_log_exp table set. When the rows
            of src carry a known positive scale c (cinv=1/c, cinv2=1/c^2),
            the stats are corrected so eps applies to the TRUE variance —
            LN is only scale-invariant when v >> eps, which fails for the
            near-zero early rows of a*."""
            stat6 = stats.tile([T, 6], f32, tag=f"{tagp}_s6")
            nc.vector.bn_stats(stat6[:], src)
            mv = stats.tile([T, 2], f32, tag=f"{tagp}_mv")
            nc.vector.bn_aggr(mv[:], stat6[:])
            if hold_pe:
                keepalive(mv[:, 0:1])
            veps = stats.tile([T, 1], f32, tag=f"{tagp}_ve")
            if cinv2 is None:
                nc.vector.tensor_scalar_add(veps[:], mv[:, 1:2], LN_EPS)
            else:
                nc.vector.tensor_scalar(veps[:], mv[:, 1:2], cinv2[:], LN_EPS,
                                        op0=ALU.mult, op1=ALU.add)
            if hold_pe:
                keepalive(veps[:])
            lv = stats.tile([T, 1], f32, tag=f"{tagp}_lv")
            nc.scalar.activation(lv[:], veps[:], AF.Ln, bias=zero_sb)
            scl = stats.tile([T, 1], f32, tag=f"{tagp}_sc")
            nc.scalar.activation(scl[:], lv[:], AF.Exp, scale=-0.5,
                                 bias=(zero_sb if nq is None else nq[:]))
            if hold_pe:
                keepalive(scl[:])
            nmr = stats.tile([T, 1], f32, tag=f"{tagp}_nr")
            nc.vector.scalar_tensor_tensor(nmr[:], mv[:, 0:1], -1.0, scl[:],
                                           op0=ALU.mult, op1=ALU.mult)
            return scl, nmr

        # ---- u = relu(emb @ Dx.T) (bf16), row sums --------------------
        u_sb = work.tile([T, N], bf16)
        su_part = stats.tile([T, NJ], f32)
        for j in range(NJ):
            ps = p_ps.tile([128, 512], f32, tag="pu", bufs=2)
            for k in range(KD):
                nc.tensor.matmul(
                    ps[:],
                    embT_sb[:, k * T:(k + 1) * T],
                    dxT_sb[:, j * 1024 + k * 512: j * 1024 + (k + 1) * 512],
                    start=(k == 0),
                    stop=(k == KD - 1),
                )
            nc.vector.tensor_scalar(
                u_sb[:, j * 512:(j + 1) * 512], ps[:], 0.0, 0.0,
                op0=ALU.max, op1=ALU.add,
                accum_out=su_part[:, j:j + 1],
            )

        # ---- C^T coefficient matrix ------------------------------------
        su = stats.tile([T, 1], f32)
        nc.vector.tensor_reduce(su[:], su_part[:], axis=AX.X, op=ALU.add)
        keepalive(su[:])
        q_sb = stats.tile([T, 1], bf16)
        nc.scalar.activation(q_sb[:], su[:], AF.Ln, scale=C2, bias=xdvec_sb)

        # ct[s,t] = exp(Q_{s-1} + trik[s,t]): the true C also carries a
        # exp(-Q_t) column factor, but that scales a* and v rows by a
        # positive per-row constant, which the downstream LayerNorms cancel
        # exactly. The exp(-Q_s) row factor of W is applied via wneg below.
        qc = p_ps.tile([T, T], f32, tag="cc", bufs=2)     # Q_{s-1} column
        nc.tensor.matmul(qc[:, 0:1], ustrict_sb, q_sb[:], start=True, stop=True)
        colsc = stats.tile([T, 1], f32)
        nc.vector.tensor_copy(colsc[:], qc[:, 0:1])
        keepalive(colsc[:])
        expo = work.tile([T, T], f32)
        nc.vector.tensor_scalar(expo[:], trik_sb[:], colsc[:], -80.0,
                                op0=ALU.add, op1=ALU.max)
        ct_sb = work.tile([T, T], bf16)               # C^T [s,t]
        nc.scalar.activation(ct_sb[:], expo[:], AF.Exp)
        qcol = stats.tile([T, 1], f32)
        nc.vector.tensor_add(qcol[:], colsc[:], q_sb[:])
        wneg = stats.tile([T, 1], f32)                # exp(-Q_s)
        nc.scalar.activation(wneg[:], qcol[:], AF.Exp, scale=-1.0,
                             bias=zero_sb)
        cinv2 = stats.tile([T, 1], f32)               # exp(-2Q_s)
        nc.scalar.activation(cinv2[:], qcol[:], AF.Exp, scale=-2.0,
                             bias=zero_sb)
        nqcol = stats.tile([T, 1], f32)               # -Q_s
        nc.vector.tensor_scalar_mul(nqcol[:], qcol[:], -1.0)

        # ---- vn = LN(emb) (off critical path) --------------------------
        vn_sb = work.tile([T, D], bf16)
        r_vn, n_vn = ln_stats(emb_sb, "vn")
        nc.gpsimd.tensor_scalar(vn_sb[:], emb_sb, r_vn[:], n_vn[:],
                                op0=ALU.mult, op1=ALU.add)

        # ---- X^T chunks = u_c @ C^T; G = X X^T, interleaved ------------
        xt_sb = work.tile([128, N], bf16)
        g = p_ps.tile([T, T], f32, tag="g", bufs=1)

        def xt_mm(c):
            tp = p_ps.tile([128, T], f32, tag="ch", bufs=3)
            nc.tensor.matmul(tp[:], u_sb[:, c * T:(c + 1) * T], ct_sb[:],
                             start=True, stop=True)
            if c % 2 == 0:
                nc.vector.tensor_copy(xt_sb[:, c * T:(c + 1) * T], tp[:])
            else:
                nc.scalar.copy(xt_sb[:, c * T:(c + 1) * T], tp[:])

        for c in range(KN):
            xt_mm(c)
            if c >= 2:
                cg = c - 2
                nc.tensor.matmul(g[:], xt_sb[:, cg * T:(cg + 1) * T],
                                 xt_sb[:, cg * T:(cg + 1) * T],
                                 start=(cg == 0), stop=False)
        for cg in (KN - 2, KN - 1):
            nc.tensor.matmul(g[:], xt_sb[:, cg * T:(cg + 1) * T],
                             xt_sb[:, cg * T:(cg + 1) * T],
                             start=False, stop=(cg == KN - 1))

        # ---- a* = (G . dmask)^T @ vn, LN, transpose --------------------
        wt_sb = work.tile([T, T], bf16)
        nc.vector.scalar_tensor_tensor(wt_sb[:], g[:], wneg[:], dmaskT_sb,
                                       op0=ALU.mult, op1=ALU.mult)
        aps = p_ps.tile([T, D], f32, tag="g", bufs=1)
        nc.tensor.matmul(aps[:], wt_sb[:], vn_sb[:], start=True, stop=True)
        r_a, n_a = ln_stats(aps[:], "la", hold_pe=True, nq=nqcol,
                            cinv2=cinv2)
        lna_sb = work.tile([T, D], bf16)
        nc.scalar.activation(lna_sb[:, 0:T], aps[:, 0:T], AF.Identity,
                             scale=r_a[:], bias=n_a[:])
        nc.vector.tensor_scalar(lna_sb[:, T:D], aps[:, T:D], r_a[:], n_a[:],
                                op0=ALU.mult, op1=ALU.add)

        lnaT_sb = work.tile([128, KD * T], bf16)
        for k in range(KD):
            tp = p_ps.tile([128, T], bf16, tag="ch", bufs=3)
            nc.tensor.transpose(tp[:], lna_sb[:, k * T:(k + 1) * T], ident_sb)
            if k % 2 == 0:
                nc.vector.tensor_copy(lnaT_sb[:, k * T:(k + 1) * T], tp[:])
            else:
                nc.scalar.copy(lnaT_sb[:, k * T:(k + 1) * T], tp[:])

        # ---- Ycore^T chunks -> Y^T = relu(.)*X^T -> v accumulation -----
        yt_sb = work.tile([128, N], bf16)
        vps = p_ps.tile([T, D], f32, tag="g", bufs=1)

        def yc_mm(c):
            yc = p_ps.tile([128, T], f32, tag="ch", bufs=3)
            for k in range(KD):
                nc.tensor.matmul(
                    yc[:],
                    dyT_sb[:, c * 256 + k * 128: c * 256 + (k + 1) * 128],
                    lnaT_sb[:, k * T:(k + 1) * T],
                    start=(k == 0), stop=(k == KD - 1),
                )
            if c % 2 == 0:
                nc.vector.scalar_tensor_tensor(
                    yt_sb[:, c * T:(c + 1) * T], yc[:], 0.0,
                    xt_sb[:, c * T:(c + 1) * T], op0=ALU.max, op1=ALU.mult,
                )
            else:
                ycr = work.tile([128, T], bf16, tag="ycr", bufs=2)
                nc.scalar.activation(ycr[:], yc[:], AF.Relu, bias=zero_sb)
                nc.gpsimd.tensor_mul(yt_sb[:, c * T:(c + 1) * T], ycr[:],
                                     xt_sb[:, c * T:(c + 1) * T])

        for c in range(KN):
            yc_mm(c)
            if c >= 2:
                cv = c - 2
                nc.tensor.matmul(vps[:], yt_sb[:, cv * T:(cv + 1) * T],
                                 eT_sb[:, cv * D:(cv + 1) * D],
                                 start=(cv == 0), stop=False)
        for cv in (KN - 2, KN - 1):
            nc.tensor.matmul(vps[:], yt_sb[:, cv * T:(cv + 1) * T],
                             eT_sb[:, cv * D:(cv + 1) * D],
                             start=False, stop=(cv == KN - 1))

        # ---- v* = LN(vps) -> out ---------------------------------------
        r_v, n_v = ln_stats(vps[:], "vs", nq=nqcol, cinv2=cinv2)
        vstar_sb = work.tile([T, D], f32)
        nc.scalar.activation(vstar_sb[:, 0:T], vps[:, 0:T], AF.Identity,
                             scale=r_v[:], bias=n_v[:])
        nc.vector.tensor_scalar(vstar_sb[:, T:D], vps[:, T:D], r_v[:], n_v[:],
                                op0=ALU.mult, op1=ALU.add)
        nc.sync.dma_start(d_out[:], vstar_sb[:])

    # The const-AP pool memsets are the first *named* instructions and
    # nothing references them anymore (all activation biases are explicit
    # APs) — delete the dead code so the program truly starts at the DMA
    # posts.
    used = set()
    for f in nc.m.functions:
        for bb in f.blocks:
            for ins in bb.instructions:
                for a in list(ins.ins) + list(ins.outs):
                    n = getattr(a, "tensor_name", None) or getattr(
                        getattr(a, "memory_location", None), "tensor_name", None)
                    if n:
                        used.add(n)
    for f in nc.m.functions:
        for bb in f.blocks:
            keep = []
            for ins in bb.instructions:
                if ins.opcode == "Memset":
                    outs = [getattr(a, "tensor_name", "") or "" for a in ins.outs]
                    if any(o.startswith("const-") for o in outs):
                        continue
                keep.append(ins)
            bb.instructions = keep
    return _split_multiwait(nc, mybir)


def _numpy_fallback(embeddings, E, Dx, Dy, x_state, rho_state):
    # General-path reference (only used if initial states are nonzero).
    def ln(x):
        m = x.mean(-1, keepdims=True)
        v = ((x - m) ** 2).mean(-1, keepdims=True)
        return (x - m) / np.sqrt(v + LN_EPS)

    x_s = x_state.astype(np.float32).copy()
    rho = rho_state.astype(np.float32).copy()
    outs = np.zeros((B, T, D), dtype=np.float32)
    for t in range(T):
        v_prev = embeddings[:, t, :]
        x_upd = np.maximum(v_prev @ Dx.T, 0.0)
        x_t = XD * x_s + x_upd
        x_t = x_t / np.maximum(np.abs(x_t).sum(-1, keepdims=True), 1e-12)
        a_star = np.einsum("bdn,bn->bd", rho, x_t)
        y_core = ln(a_star) @ Dy.T
        y_t = np.maximum(y_core, 0.0) * np.maximum(x_t, 0.0)
        outs[:, t, :] = ln(y_t @ E.T)
        vn = ln(v_prev)
        rho = UD * rho + np.einsum("bd,bn->bdn", vn, x_t)
        x_s = x_t
    return outs


def kernel(embeddings, E, Dx, Dy, x_state, rho_state):
    import ml_dtypes

    embeddings = np.ascontiguousarray(embeddings, dtype=np.float32)
    E = np.ascontiguousarray(E, dtype=np.float32)
    Dx = np.ascontiguousarray(Dx, dtype=np.float32)
    Dy = np.ascontiguousarray(Dy, dtype=np.float32)

    if np.any(x_state) or np.any(rho_state):
        return _numpy_fallback(embeddings, E, Dx, Dy,
                               np.asarray(x_state, np.float32),
                               np.asarray(rho_state, np.float32))

    from concourse.bass_utils import run_bass_kernel_spmd

    if "nc" not in _cache:
        _cache["nc"] = _build()
    nc = _cache["nc"]

    bf = ml_dtypes.bfloat16
    consts_bf = _consts_bf16()
    cf32 = _consts_f32()
    # SBUF-layout packing: row p holds that partition's contiguous span.
    dxT = np.ascontiguousarray(
        Dx.T.reshape(KD, 128, NJ, 512).transpose(1, 2, 0, 3).reshape(128, KD * N)
    ).astype(bf)
    dyT = np.ascontiguousarray(
        Dy.T.reshape(KD, 128, KN, 128).transpose(1, 2, 0, 3).reshape(128, KD * N)
    ).astype(bf)
    eT = np.ascontiguousarray(
        E.T.reshape(KN, 128, D).transpose(1, 0, 2).reshape(128, KN * D)
    ).astype(bf)

    in_maps = []
    for b in range(B):
        emb_b = embeddings[b]
        embT_b = np.ascontiguousarray(
            emb_b.T.reshape(KD, 128, T).transpose(1, 0, 2).reshape(128, KD * T)
        ).astype(bf)
        blob = np.concatenate(
            [consts_bf, embT_b, emb_b.astype(bf), dxT, dyT, eT], axis=1)
        assert blob.shape == (128, CB), blob.shape
        in_maps.append({"blob": np.ascontiguousarray(blob), "cf32": cf32})

    res = run_bass_kernel_spmd(nc, in_maps, list(range(B)))
    _cache["last_results"] = res
    return np.stack([res.results[i]["out"] for i in range(B)])
